# revision 1
# baseline (speedup 1.0000x reference)
"""Trainium2 Bass kernel for nn_CausalVideoAttention (b=2, s=2048, d=512, 8 heads).

Sharding: 8 cores = (batch, head-pair): core c -> batch c//4, heads {2*(c%4), 2*(c%4)+1}.
Each core computes qkv projection for its head pair, causal attention over the full
sequence, and a partial output projection (its heads' slice of Wo), producing
out_core = (z_pair @ Wo_pair^T)^T  as a [512, 2048] tensor. Host sums the 4 cores of
each batch and transposes. No device collectives needed.

On-chip layout is "transposed domain": activations stored [feature, seq] so every
matmul has contraction on partitions and free dim 512 (full-rate fp32r).
"""

import sys

for _p in ("/opt/trn_rl_repo",):
    if _p not in sys.path:
        sys.path.insert(0, _p)

import numpy as np
import concourse.bass as bass
import concourse.mybir as mybir
import concourse.tile as tile
from concourse import bacc
from concourse.bass_utils import run_bass_kernel_spmd
from concourse.dve_ops import (
    RECIPROCAL_APPROX_FAST,
    RECIPROCAL_APPROX_NR,
    RECIP_APPROX_FAST_CONSTS,
)

F32 = mybir.dt.float32
F32R = mybir.dt.float32r
AF = mybir.ActivationFunctionType

B, S, D = 2, 2048, 512
NH, DH = 8, 64
P = 128          # partitions / tile edge
NT = S // P      # 16 s-tiles
NCH = D // P     # 4 din chunks
QC = 512         # q-chunk width
NQC = S // QC    # 4 q-chunks
EPS = 1e-6
NEG = -30000.0

_CACHE = {}


def _build_program():
    nc = bacc.Bacc("TRN2", target_bir_lowering=False, debug=False, num_devices=8)
    xb = nc.dram_tensor("xb", [S, D], F32R, kind="ExternalInput").ap()
    wq = nc.dram_tensor("wq", [P, D], F32R, kind="ExternalInput").ap()
    wk = nc.dram_tensor("wk", [P, D], F32R, kind="ExternalInput").ap()
    wv = nc.dram_tensor("wv", [P, D], F32R, kind="ExternalInput").ap()
    wo = nc.dram_tensor("wo", [D, P], F32R, kind="ExternalInput").ap()
    idn = nc.dram_tensor("idn", [P, P], F32R, kind="ExternalInput").ap()
    out = nc.dram_tensor("out", [D, S], F32, kind="ExternalOutput").ap()

    with tile.TileContext(nc) as tc:
        with tc.tile_pool(name="const", bufs=1) as cpool, \
             tc.tile_pool(name="big", bufs=1) as big:
            # ---- constants ----
            ident = cpool.tile([P, P], F32R, tag="ident")
            nc.sync.dma_start(ident[:], idn[:])
            tri1 = cpool.tile([P, P], F32, tag="tri1")
            nc.vector.memset(tri1[:], 0.0)
            # keep where qf - p >= 0 (strict upper-left of diag masked)
            nc.gpsimd.affine_select(out=tri1[:], in_=tri1[:],
                                    compare_op=mybir.AluOpType.is_ge, fill=NEG,
                                    base=0, channel_multiplier=-1, pattern=[[1, P]])
            tri2 = cpool.tile([P, 256], F32, tag="tri2")
            nc.vector.memset(tri2[:], 0.0)
            # keep where qf - p - 128 >= 0 (first 128 cols fully masked + triangle)
            nc.gpsimd.affine_select(out=tri2[:], in_=tri2[:],
                                    compare_op=mybir.AluOpType.is_ge, fill=NEG,
                                    base=-P, channel_multiplier=-1, pattern=[[1, 256]])
            sel = cpool.tile([P, 2], F32, tag="sel")
            nc.vector.memset(sel[:], 0.0)
            nc.vector.memset(sel[0:64, 0:1], 1.0)
            nc.vector.memset(sel[64:128, 1:2], 1.0)

            # ---- persistent big tiles ----
            xT = [big.tile([P, S], F32R, name=f"xT{c}", tag=f"xT{c}") for c in range(NCH)]
            qTr = big.tile([P, S], F32, tag="qTr")      # raw q^T (pre-norm)
            qTn = big.tile([P, S], F32R, tag="qTn")     # normalized q^T
            kT = big.tile([P, S], F32R, tag="kT")
            vT = big.tile([P, S], F32R, tag="vT")
            vaug0 = big.tile([P, 65 * NT], F32R, tag="vaug0")
            vaug1 = big.tile([P, P * NT], F32R, tag="vaug1")
            wqT = big.tile([P, D], F32R, tag="wqT")
            wkT = big.tile([P, D], F32R, tag="wkT")
            wvT = big.tile([P, D], F32R, tag="wvT")
            woT = big.tile([P, D], F32R, tag="woT")
            sqq = big.tile([P, S], F32, tag="sqq")
            sqk = big.tile([P, S], F32, tag="sqk")
            fqT = [big.tile([1, S], F32, name=f"fqT{h}", tag=f"fqT{h}") for h in range(2)]
            fqb = [big.tile([P, S], F32, name=f"fqb{h}", tag=f"fqb{h}") for h in range(2)]
            frec = big.tile([P, 64], F32, tag="frec")

            # ================= phase 1: load & transpose =================
            with tc.tile_pool(name="xin", bufs=3) as xin, \
                 tc.tile_pool(name="tps", bufs=4, space="PSUM") as tps:
                # weights in, transpose to W^T chunks
                for w_in, wT in ((wq, wqT), (wk, wkT), (wv, wvT)):
                    wsb = xin.tile([P, D], F32R, tag="wsb")
                    nc.sync.dma_start(wsb[:], w_in[:])
                    for c in range(NCH):
                        pt = tps.tile([P, P], F32R, tag="tp")
                        nc.tensor.transpose(pt[:], wsb[:, P * c:P * (c + 1)], ident[:])
                        nc.vector.tensor_copy(wT[:, P * c:P * (c + 1)], pt[:])
                # wo arrives [512,128]; view as [128, (c,128)] then transpose chunks
                wosb = xin.tile([P, D], F32R, tag="wsb")
                nc.sync.dma_start(wosb[:].rearrange("p (c j) -> p c j", c=NCH),
                                  wo.rearrange("(c p) j -> p c j", p=P))
                for c in range(NCH):
                    pt = tps.tile([P, P], F32R, tag="tp")
                    nc.tensor.transpose(pt[:], wosb[:, P * c:P * (c + 1)], ident[:])
                    nc.vector.tensor_copy(woT[:, P * c:P * (c + 1)], pt[:])

                for t in range(NT):
                    xsb = xin.tile([P, D], F32R, tag="xsb")
                    nc.sync.dma_start(xsb[:], xb[P * t:P * (t + 1), :])
                    for c in range(NCH):
                        pt = tps.tile([P, P], F32R, tag="tp")
                        nc.tensor.transpose(pt[:], xsb[:, P * c:P * (c + 1)], ident[:])
                        dst = xT[c][:, P * t:P * (t + 1)]
                        if c >= 2:
                            nc.scalar.copy(dst, pt[:])
                        else:
                            nc.vector.tensor_copy(dst, pt[:])

            # ================= phase 2: projections + rmsnorm =================
            with tc.tile_pool(name="pps", bufs=3, space="PSUM") as pps, \
                 tc.tile_pool(name="ssps", bufs=1, space="PSUM") as ssps, \
                 tc.tile_pool(name="fps", bufs=1, space="PSUM") as fps, \
                 tc.tile_pool(name="vps", bufs=2, space="PSUM") as vps:
                ss = ssps.tile([P, 64], F32, tag="ss")
                for sc in range(NQC):
                    qs = slice(QC * sc, QC * (sc + 1))
                    for name, wT, in2 in (("q", wqT, None), ("k", wkT, None), ("v", wvT, None)):
                        ps = pps.tile([P, QC], F32, tag="proj")
                        for c in range(NCH):
                            nc.tensor.matmul(ps[:], wT[:, P * c:P * (c + 1)],
                                             xT[c][:, qs], start=(c == 0), stop=(c == 3))
                        if name == "q":
                            nc.vector.tensor_copy(qTr[:, qs], ps[:])
                            nc.scalar.square(sqq[:, qs], ps[:])
                        elif name == "k":
                            nc.vector.tensor_copy(kT[:, qs], ps[:])
                            nc.scalar.square(sqk[:, qs], ps[:])
                        else:
                            nc.vector.tensor_copy(vT[:, qs], ps[:])
                    # sum-of-squares per head via selector matmul (plain fp32, exact)
                    for tl in range(4):
                        t = 4 * sc + tl
                        nc.tensor.matmul(ss[:, 2 * t:2 * t + 2],
                                         sqq[:, P * t:P * (t + 1)], sel[:],
                                         start=True, stop=True)
                        nc.tensor.matmul(ss[:, 32 + 2 * t:32 + 2 * t + 2],
                                         sqk[:, P * t:P * (t + 1)], sel[:],
                                         start=True, stop=True)

                # factors per chunk: 1 / (sqrt(ss/64) + eps)  [s-native orientation]
                for sc in range(NQC):
                    qs = slice(QC * sc, QC * (sc + 1))
                    for base in (0, 32):
                        cs = slice(base + 8 * sc, base + 8 * sc + 8)
                        nc.scalar.activation(frec[:, cs], ss[:, cs], AF.Sqrt,
                                             bias=0.0, scale=1.0 / DH)
                        nc.vector.tensor_scalar_add(frec[:, cs], frec[:, cs], EPS)
                        nc.vector.reciprocal(frec[:, cs], frec[:, cs])
                    for tl in range(4):
                        t = 4 * sc + tl
                        for h in range(2):
                            fp = fps.tile([1, P], F32, tag="fp")
                            nc.tensor.transpose(fp[:], frec[:, 2 * t + h:2 * t + h + 1],
                                                ident[:].bitcast(F32))
                            nc.vector.tensor_copy(fqT[h][:, P * t:P * (t + 1)], fp[:])
                    for h in range(2):
                        nc.gpsimd.partition_broadcast(fqb[h][:, qs], fqT[h][:, qs])
                    nc.vector.tensor_mul(qTn[0:64, qs], qTr[0:64, qs], fqb[0][0:64, qs])
                    nc.vector.tensor_mul(qTn[64:128, qs], qTr[64:128, qs], fqb[1][64:128, qs])

                # v back to native orientation, augmented with ones column
                nc.vector.memset(vaug0[:, 64::65].bitcast(F32), 1.0)
                nc.vector.memset(vaug1[:].bitcast(F32), 0.0)
                nc.vector.memset(vaug1[:, 0::P].bitcast(F32), 1.0)
                for t in range(NT):
                    vp = vps.tile([P, P], F32R, tag="vp")
                    nc.tensor.transpose(vp[:], vT[:, P * t:P * (t + 1)], ident[:])
                    nc.vector.tensor_copy(vaug0[:, 65 * t:65 * t + 64], vp[:, 0:64])
                    nc.vector.tensor_copy(vaug1[:, P * t + 64:P * t + 128], vp[:, 64:128])

            # ================= phase 3: attention + output =================
            with tc.tile_pool(name="scps", bufs=4, space="PSUM") as scps, \
                 tc.tile_pool(name="ztps", bufs=2, space="PSUM") as ztps, \
                 tc.tile_pool(name="ops", bufs=2, space="PSUM") as ops, \
                 tc.tile_pool(name="att", bufs=6) as att, \
                 tc.tile_pool(name="nrm", bufs=2) as nrm:
                C = RECIP_APPROX_FAST_CONSTS
                for j in range(NQC):
                    q0 = QC * j
                    zTn = nrm.tile([P, QC], F32R, tag="zTn")
                    nkb = 4 * j + 4
                    for h in range(2):
                        hs = slice(64 * h, 64 * (h + 1))
                        zt = ztps.tile([P, QC], F32, tag="zt")
                        for kb in range(nkb):
                            i = kb - 4 * j
                            qoff = 0
                            if i >= 0:
                                qoff = 256 if i == 3 else P * i
                            fr = QC - qoff
                            sc_ps = scps.tile([P, fr], F32, tag="sc")
                            nc.tensor.matmul(sc_ps[:],
                                             kT[hs, P * kb:P * (kb + 1)],
                                             qTn[hs, q0 + qoff:q0 + QC],
                                             start=True, stop=True)
                            if i == 3:
                                nc.vector.tensor_add(sc_ps[:, 0:256], sc_ps[:, 0:256], tri2[:])
                            elif i >= 0:
                                nc.vector.tensor_add(sc_ps[:, 0:P], sc_ps[:, 0:P], tri1[:])
                            eT = att.tile([P, fr], F32R, tag="eT")
                            nc.scalar.activation(eT[:], sc_ps[:], AF.Exp, bias=0.0,
                                                 scale=frec[:, 32 + 2 * kb + h:33 + 2 * kb + h])
                            if h == 0:
                                nc.tensor.matmul(zt[0:65, qoff:QC],
                                                 vaug0[:, 65 * kb:65 * kb + 65], eT[:],
                                                 start=(kb == 0), stop=(kb == nkb - 1))
                            else:
                                nc.tensor.matmul(zt[:, qoff:QC],
                                                 vaug1[:, P * kb:P * (kb + 1)], eT[:],
                                                 start=(kb == 0), stop=(kb == nkb - 1))
                        # softmax denominators: h0 row at p64, h1 row at p0
                        dp = 64 if h == 0 else 0
                        ds = slice(dp, dp + 1)
                        dra = nrm.tile([P, QC], F32, tag="dra")
                        drb = nrm.tile([P, QC], F32, tag="drb")
                        nc.vector.tensor_copy(dra[ds, :], zt[ds, :])
                        nc.vector.reciprocal(drb[ds, :], dra[ds, :])
                        if h == 0:
                            d0 = nrm.tile([1, QC], F32, tag="d0")
                            nc.sync.dma_start(d0[:], drb[64:65, :])
                            src_row = d0[:]
                        else:
                            src_row = drb[0:1, :]
                        db = nrm.tile([P, QC], F32, tag="db")
                        nc.gpsimd.partition_broadcast(db[:], src_row)
                        zrows = zt[0:64, :] if h == 0 else zt[64:128, :]
                        nc.vector.tensor_mul(zTn[hs, :], zrows, db[hs, :])
                    for dc in range(NCH):
                        op = ops.tile([P, QC], F32, tag="op")
                        nc.tensor.matmul(op[:], woT[:, P * dc:P * (dc + 1)], zTn[:],
                                         start=True, stop=True)
                        osb = att.tile([P, QC], F32, tag="osb")
                        if dc % 2 == 0:
                            nc.vector.tensor_copy(osb[:], op[:])
                        else:
                            nc.scalar.copy(osb[:], op[:])
                        nc.sync.dma_start(out[P * dc:P * (dc + 1), q0:q0 + QC], osb[:])

    nc.finalize()
    return nc


def _numpy_reference(x, Wqkv, Wo, scale_q, scale_k, mask):
    b, s, d = x.shape
    dh = d // NH
    qkv = x @ Wqkv.T
    q, k, v = np.split(qkv, 3, axis=-1)

    def rms(t, scale):
        r = np.sqrt(np.mean(np.square(t), axis=-1, keepdims=True)) + EPS
        return t / r * scale

    q = rms(q.reshape(b, s, NH, dh), scale_q)
    k = rms(k.reshape(b, s, NH, dh), scale_k)
    v = v.reshape(b, s, NH, dh)
    attn = np.einsum('bqhd,bkhd->bhqk', q, k)
    attn = np.where(mask[None, None], -np.inf, attn)
    attn = attn - attn.max(axis=-1, keepdims=True)
    p = np.exp(attn)
    p = p / p.sum(axis=-1, keepdims=True)
    z = np.einsum('bhqk,bkhd->bqhd', p, v).reshape(b, s, d)
    return (z @ Wo.T).astype(np.float32)


def kernel(x, Wqkv, Wo, scale_q, scale_k, mask):
    x = np.asarray(x, np.float32)
    Wqkv = np.asarray(Wqkv, np.float32)
    Wo = np.asarray(Wo, np.float32)
    causal = np.triu(np.ones((S, S), dtype=bool), k=1)
    if (not np.allclose(np.asarray(scale_q), 1.0) or
            not np.allclose(np.asarray(scale_k), 1.0) or
            not np.array_equal(np.asarray(mask), causal) or
            x.shape != (B, S, D)):
        return _numpy_reference(x, Wqkv, Wo, np.asarray(scale_q), np.asarray(scale_k),
                                np.asarray(mask))

    if "nc" not in _CACHE:
        _CACHE["nc"] = _build_program()
    nc = _CACHE["nc"]

    in_maps = []
    for core in range(8):
        bb = core // 4
        h0 = 2 * (core % 4)
        r = slice(DH * h0, DH * h0 + P)
        in_maps.append({
            "xb": np.ascontiguousarray(x[bb]),
            "wq": np.ascontiguousarray(Wqkv[r, :]),
            "wk": np.ascontiguousarray(Wqkv[D:][r, :]),
            "wv": np.ascontiguousarray(Wqkv[2 * D:][r, :]),
            "wo": np.ascontiguousarray(Wo[:, r]),
            "idn": np.eye(P, dtype=np.float32),
        })
    _CACHE["last_in_maps"] = in_maps
    res = run_bass_kernel_spmd(nc, in_maps, core_ids=list(range(8)))
    outp = np.zeros((B, S, D), np.float32)
    for core in range(8):
        outp[core // 4] += res.results[core]["out"].T
    return outp



# revision 34
# speedup vs baseline: 136.7172x; 136.7172x over previous
"""Trainium2 Bass kernel for nn_CausalVideoAttention (b=2, s=2048, d=512, 8 heads).

Sharding: 8 cores = (batch, head-pair): core c -> batch c//4, heads {2*(c%4), 2*(c%4)+1}.
Each core computes the qkv projection for its head pair (fp16 inputs, f32 psum),
rms-normalizes q and k (factors folded in on-chip), runs causal attention over the
full sequence, and returns normalized per-pair z in native [seq, dh*2] orientation
(fp16). The host applies the output projection Wo (one sgemm per batch) and
concatenates head pairs. No device collectives.

On-chip layout: activations stored [feature, seq] ("transposed domain") so every
projection/score matmul contracts over partitions; the z accumulation is done with
q on partitions (out free = 65 = dh+denominator) which halves PE row count vs the
wide-free form and makes softmax normalization a per-partition scalar multiply.
"""

import sys

for _p in ("/opt/trn_rl_repo",):
    if _p not in sys.path:
        sys.path.insert(0, _p)

import numpy as np

B, S, D = 2, 2048, 512
NH, DH = 8, 64
P = 128          # partitions / tile edge
NCH = D // P     # 4 din chunks
QC = 512         # q-chunk width
NQC = S // QC    # 4 q-chunks
NT = S // P      # 16 s-tiles
EPS = 1e-6

_CACHE = {}


def _build_program(debug_taps=False):
    import concourse.bass as bass  # noqa: F401
    import concourse.mybir as mybir
    import concourse.tile as tile
    from concourse import bacc

    F32 = mybir.dt.float32
    F32R = mybir.dt.float32r
    F16 = mybir.dt.float16
    BF16 = mybir.dt.bfloat16
    AF = mybir.ActivationFunctionType

    nc = bacc.Bacc("TRN2", target_bir_lowering=False, debug=False, num_devices=8)
    xt = nc.dram_tensor("xt", [P, NCH * S], F16, kind="ExternalInput").ap()
    wt = nc.dram_tensor("wt", [P, NCH * 3 * P], F16, kind="ExternalInput").ap()
    z = nc.dram_tensor("z", [S, P], F16, kind="ExternalOutput").ap()
    taps = {}
    if debug_taps:
        taps["dq"] = nc.dram_tensor("dq", [P, S], F16, kind="ExternalOutput").ap()
        taps["dk"] = nc.dram_tensor("dk", [P, S], F16, kind="ExternalOutput").ap()
        taps["dv0"] = nc.dram_tensor("dv0", [P, 66 * NT], F32, kind="ExternalOutput").ap()
        taps["dv1"] = nc.dram_tensor("dv1", [P, 66 * NT], F32, kind="ExternalOutput").ap()
        for kk in range(4):
            taps[f"de{kk}"] = nc.dram_tensor(f"de{kk}", [P, 1024], F32, kind="ExternalOutput").ap()
        taps["dzt"] = nc.dram_tensor("dzt", [P, 264], F32, kind="ExternalOutput").ap()

    with tile.TileContext(nc) as tc:
        with tc.tile_pool(name="const", bufs=1) as cpool, \
             tc.tile_pool(name="big", bufs=1) as big:
            # selector [128, 2]: col h sums partitions of head h (for sum-of-squares)
            sel2 = cpool.tile([P, 2], F16, tag="sel2")
            nc.vector.memset(sel2[:], 0.0)
            nc.vector.memset(sel2[0:64, 0:1], 1.0)
            nc.vector.memset(sel2[64:128, 1:2], 1.0)
            # broadcast selector [2, 128]: bsel[p, i] = 1 iff i // 64 == p,
            # built as ones gated by two affine selects (64p <= i < 64p+64)
            bsel = cpool.tile([2, P], F32R, tag="bsel")
            bselt = cpool.tile([2, P], F32, tag="bselt")
            nc.vector.memset(bselt[:], 1.0)
            nc.gpsimd.affine_select(out=bselt[:], in_=bselt[:],
                                    compare_op=mybir.AluOpType.is_ge, fill=0.0,
                                    base=0, channel_multiplier=-64, pattern=[[1, P]])
            nc.gpsimd.affine_select(out=bselt[:], in_=bselt[:],
                                    compare_op=mybir.AluOpType.is_ge, fill=0.0,
                                    base=63, channel_multiplier=64, pattern=[[-1, P]])
            nc.gpsimd.tensor_copy(bsel[:], bselt[:])

            # persistent tiles
            xts = big.tile([P, NCH * S], F16, tag="xts")        # x^T din-chunks
            wts = big.tile([P, NCH * 3 * P], F16, tag="wts")    # w^T din-chunks
            qTn = big.tile([P, S], F16, tag="qTn")              # normalized q^T
            kTn = big.tile([P, S], F16, tag="kTn")              # normalized k^T
            # v augmented with a ones column per s-tile: [v_h (64) | 1]
            vaug = [big.tile([P, 66 * NT], F32R, name=f"vaug{h}", tag=f"vaug{h}")
                    for h in range(2)]

            for h in range(2):
                nc.vector.memset(vaug[h][:].bitcast(F32), 1.0)
            # one plain 2D DMA per input (host supplies chunk-major layout):
            # a single queue/completion event per tensor keeps every
            # consumer's wait identical, so nothing dispatches out of order
            nc.sync.dma_start(xts[:], xt[:])
            nc.sync.dma_start(wts[:], wt[:])

            # ============ phase 1: projections + rmsnorm factors ============
            with tc.tile_pool(name="pps", bufs=2, space="PSUM") as pps, \
                 tc.tile_pool(name="sps", bufs=1, space="PSUM") as sps, \
                 tc.tile_pool(name="fps", bufs=1, space="PSUM") as fps, \
                 tc.tile_pool(name="vps", bufs=2, space="PSUM") as vps, \
                 tc.tile_pool(name="sq", bufs=2) as sqp, \
                 tc.tile_pool(name="fr", bufs=2) as frp:
                for sc in range(NQC):
                    qs = slice(QC * sc, QC * (sc + 1))
                    ssqk = sps.tile([2, 2 * QC], F32, tag="ssqk")
                    sq = sqp.tile([P, 2 * QC], F16, tag="sq")
                    raws = []
                    for ki, w0 in ((0, 0), (1, P)):  # q then k
                        ps = pps.tile([P, QC], F32, tag="proj")
                        for c in range(NCH):
                            nc.tensor.matmul(
                                ps[:],
                                wts[:, 3 * P * c + w0:3 * P * c + w0 + P],
                                xts[:, S * c + QC * sc:S * c + QC * (sc + 1)],
                                start=(c == 0), stop=(c == 3))
                        raw = sqp.tile([P, QC], F16, tag="raw")
                        nc.vector.tensor_copy(raw[:], ps[:])
                        nc.gpsimd.tensor_mul(sq[:, QC * ki:QC * (ki + 1)],
                                             raw[:], raw[:])
                        nc.tensor.matmul(ssqk[:, QC * ki:QC * (ki + 1)], sel2[:],
                                         sq[:, QC * ki:QC * (ki + 1)],
                                         start=True, stop=True)
                        raws.append(raw)
                    # factors: 1/sqrt(mean sq); rows = heads, cols = [fq | fk]
                    srt = frp.tile([2, 2 * QC], F32, tag="srt")
                    ftmp = frp.tile([2, 2 * QC], F32, tag="ftmp")
                    frec = frp.tile([2, 2 * QC], F32R, tag="frec")
                    nc.scalar.activation(srt[:], ssqk[:], AF.Sqrt,
                                         bias=0.0, scale=1.0 / DH)
                    nc.vector.reciprocal_approx_fast(ftmp[:], srt[:])
                    nc.gpsimd.tensor_copy(frec[:], ftmp[:])
                    # broadcast factor rows across dh partitions via PE
                    for ki, dst in ((0, qTn), (1, kTn)):
                        fb = fps.tile([P, QC], F32, tag="fb")
                        nc.tensor.matmul(fb[:], bsel[:],
                                         frec[:, QC * ki:QC * (ki + 1)],
                                         start=True, stop=True)
                        nc.vector.tensor_mul(dst[:, qs], raws[ki][:], fb[:])
                    # v in native orientation [s-pos, dh] with ones column
                    psv = vps.tile([P, QC], F32, tag="psv")
                    for tl in range(4):
                        t = 4 * sc + tl
                        for c in range(NCH):
                            nc.tensor.matmul(
                                psv[:, P * tl:P * (tl + 1)],
                                xts[:, S * c + P * t:S * c + P * (t + 1)],
                                wts[:, 3 * P * c + 2 * P:3 * P * (c + 1)],
                                start=(c == 0), stop=(c == 3))
                    for h in range(2):
                        for tl in range(4):
                            t = 4 * sc + tl
                            nc.vector.tensor_copy(
                                vaug[h][:, 66 * t:66 * t + 64],
                                psv[:, P * tl + 64 * h:P * tl + 64 * (h + 1)])

            # ================= phase 2: attention =================
            with tc.tile_pool(name="scps", bufs=2, space="PSUM") as scps, \
                 tc.tile_pool(name="ztps", bufs=2, space="PSUM") as ztps, \
                 tc.tile_pool(name="att", bufs=18) as att, \
                 tc.tile_pool(name="nrm", bufs=2) as nrm, \
                 tc.tile_pool(name="zo", bufs=2) as zop:
                for j in range(NQC):
                    q0 = QC * j
                    nkb = 4 * j + 4
                    zts = [ztps.tile([P, 264], F32, name=f"zt{j}_{h}", tag=f"zt{h}")
                           for h in range(2)]
                    eTs = []
                    for kb in range(nkb):
                        i = kb - 4 * j
                        qoff = max(0, P * i)
                        fr = QC - qoff
                        # head h occupies cols [512h : 512h + fr] so each
                        # matmul output stays inside one 512-float psum bank
                        sc_ps = scps.tile([P, 1024], F32, tag="sc")
                        eT = att.tile([P, 1024], F32R, tag="eT")
                        for h in range(2):
                            hs = slice(64 * h, 64 * (h + 1))
                            nc.tensor.matmul(sc_ps[:, QC * h:QC * h + fr],
                                             kTn[hs, P * kb:P * (kb + 1)],
                                             qTn[hs, q0 + qoff:q0 + QC],
                                             start=True, stop=True)
                        both = slice(0, QC + fr) if fr == QC else None
                        if both is not None:
                            nc.scalar.activation(eT[:, both], sc_ps[:, both],
                                                 AF.Exp, bias=0.0, scale=1.0)
                        else:
                            ap_o = eT[:].rearrange("p (h f) -> p h f", h=2)[:, :, 0:fr]
                            ap_i = sc_ps[:].rearrange("p (h f) -> p h f", h=2)[:, :, 0:fr]
                            nc.scalar.activation(ap_o, ap_i,
                                                 AF.Exp, bias=0.0, scale=1.0)
                        if i >= 0:
                            # zero the strictly-upper triangle of the diagonal
                            # 128-col block of each head (keep where col >= p)
                            tri = eT[:].rearrange("p (h f) -> p h f", h=2)[:, :, 0:P]
                            nc.gpsimd.affine_select(
                                out=tri, in_=tri,
                                compare_op=mybir.AluOpType.is_ge, fill=0.0,
                                base=0, channel_multiplier=-1,
                                pattern=[[0, 2], [1, P]])
                        if debug_taps and j == 0:
                            nc.sync.dma_start(taps[f"de{kb}"], eT[:].bitcast(F32))
                        eTs.append((kb, qoff, fr, eT))
                    # z accumulation: q on partitions, free = [v 64 | denom].
                    # qsub-major so each psum region's accumulation group
                    # closes before the next one starts (a psum bank holds
                    # only one open accumulation group at a time).
                    for qsub in range(4):
                        for h in range(2):
                            for kb, qoff, fr, eT in eTs[:4 * j + qsub + 1]:
                                if P * qsub < qoff:
                                    continue
                                nc.tensor.matmul(
                                    zts[h][:, 66 * qsub:66 * (qsub + 1)],
                                    eT[:, QC * h + P * qsub - qoff:
                                          QC * h + P * qsub - qoff + P],
                                    vaug[h][:, 66 * kb:66 * (kb + 1)],
                                    start=(kb == 0), stop=(kb == 4 * j + qsub))
                    zsb = zop.tile([P, 4 * P], F16, tag="zsb")
                    for h in range(2):
                        rcp = nrm.tile([P, 4], F32, tag="rcp")
                        nc.vector.reciprocal(rcp[:], zts[h][:, 64::66])
                        for qsub in range(4):
                            nc.vector.tensor_scalar(
                                out=zsb[:, P * qsub + 64 * h:P * qsub + 64 * (h + 1)],
                                in0=zts[h][:, 66 * qsub:66 * qsub + 64],
                                scalar1=rcp[:, qsub:qsub + 1],
                                scalar2=None,
                                op0=mybir.AluOpType.mult)
                    nc.sync.dma_start(
                        z[q0:q0 + QC, :].rearrange("(qs p) c -> p qs c", p=P),
                        zsb[:].rearrange("p (qs c) -> p qs c", qs=4))
                    if debug_taps and j == 0:
                        zcopy = zop.tile([P, 264], F32, tag="zcopy")
                        nc.vector.tensor_copy(zcopy[:], zts[0][:])
                        nc.sync.dma_start(taps["dzt"], zcopy[:])
            if debug_taps:
                nc.sync.dma_start(taps["dq"], qTn[:])
                nc.sync.dma_start(taps["dk"], kTn[:])
                nc.sync.dma_start(taps["dv0"], vaug[0][:].bitcast(F32))
                nc.sync.dma_start(taps["dv1"], vaug[1][:].bitcast(F32))

    nc.finalize()
    return nc


def _numpy_reference(x, Wqkv, Wo, scale_q, scale_k, mask):
    b, s, d = x.shape
    dh = d // NH
    qkv = x @ Wqkv.T
    q, k, v = np.split(qkv, 3, axis=-1)

    def rms(t, scale):
        r = np.sqrt(np.mean(np.square(t), axis=-1, keepdims=True)) + EPS
        return t / r * scale

    q = rms(q.reshape(b, s, NH, dh), scale_q)
    k = rms(k.reshape(b, s, NH, dh), scale_k)
    v = v.reshape(b, s, NH, dh)
    attn = np.einsum('bqhd,bkhd->bhqk', q, k)
    attn = np.where(mask[None, None], -np.inf, attn)
    attn = attn - attn.max(axis=-1, keepdims=True)
    p = np.exp(attn)
    p = p / p.sum(axis=-1, keepdims=True)
    zz = np.einsum('bhqk,bkhd->bqhd', p, v).reshape(b, s, d)
    return (zz @ Wo.T).astype(np.float32)


class _Runner:
    """Caches the jitted shard_map executable for nc across calls."""

    def __init__(self, nc, n_cores=8):
        import jax
        import concourse.mybir as mybir
        from jax.sharding import Mesh, PartitionSpec
        from jax.experimental.shard_map import shard_map
        from concourse.bass2jax import (
            _bass_exec_p, install_neuronx_cc_hook, partition_id_tensor)

        install_neuronx_cc_hook()
        self.nc = nc
        self.n_cores = n_cores
        partition_name = nc.partition_id_tensor.name if nc.partition_id_tensor else None
        in_names, out_names, out_avals, zero_shapes = [], [], [], []
        for alloc in nc.m.functions[0].allocations:
            if not isinstance(alloc, mybir.MemoryLocationSet):
                continue
            name = alloc.memorylocations[0].name
            if alloc.kind == "ExternalInput":
                if name != partition_name:
                    in_names.append(name)
            elif alloc.kind == "ExternalOutput":
                out_names.append(name)
                shape = tuple(alloc.tensor_shape)
                dtype = mybir.dt.np(alloc.dtype)
                out_avals.append(jax.core.ShapedArray(shape, dtype))
                zero_shapes.append((shape, dtype))
        self.in_names = in_names
        self.out_names = out_names
        self.zero_shapes = zero_shapes
        n_params = len(in_names)
        n_outs = len(out_avals)
        in_names_all = in_names + out_names + ([partition_name] if partition_name else [])
        donate = tuple(range(n_params, n_params + n_outs))

        def _body(*args):
            operands = list(args)
            if partition_name is not None:
                operands.append(partition_id_tensor())
            outs = _bass_exec_p.bind(
                *operands, out_avals=tuple(out_avals),
                in_names=tuple(in_names_all), out_names=tuple(out_names),
                lowering_input_output_aliases=(),
                sim_require_finite=True, sim_require_nnan=True, nc=nc)
            return tuple(outs)

        devices = jax.devices()[:n_cores]
        self.mesh = Mesh(np.asarray(devices), ("core",))
        in_specs = (PartitionSpec("core"),) * (n_params + n_outs)
        out_specs = (PartitionSpec("core"),) * n_outs
        self.sharded = jax.jit(
            shard_map(_body, mesh=self.mesh, in_specs=in_specs,
                      out_specs=out_specs, check_rep=False),
            donate_argnums=donate, keep_unused=True)
        self._zmaker = jax.jit(
            lambda: tuple(
                jax.numpy.zeros((n_cores * sh[0], *sh[1:]), dt)
                for sh, dt in zero_shapes),
            out_shardings=tuple(
                jax.sharding.NamedSharding(self.mesh, PartitionSpec("core"))
                for _ in zero_shapes))

    def concat_inputs(self, in_maps):
        return [np.concatenate([np.asarray(m[nm]) for m in in_maps], axis=0)
                for nm in self.in_names]

    def run_concat(self, concat_in):
        """concat_in: list of (n_cores*dim0, ...) arrays (np or device)."""
        zeros = self._zmaker()
        outs = self.sharded(*concat_in, *zeros)
        return outs

    def fetch(self, outs):
        res = []
        for c in range(self.n_cores):
            d = {}
            for i, name in enumerate(self.out_names):
                sh = self.zero_shapes[i][0]
                d[name] = np.asarray(outs[i]).reshape(self.n_cores, *sh)[c]
            res.append(d)
        return res


def _get_runner():
    if "runner" not in _CACHE:
        _CACHE["runner"] = _Runner(_build_program())
    return _CACHE["runner"]


def _prep_inputs(x, Wqkv):
    """Build per-core device inputs: fp16, transposed, din-chunk-major.

    xt: [128, 4*2048] where chunk c cols hold x[b].T rows 128c:128c+128.
    wt: [128, 4*384]  where chunk c holds [WqT | WkT | WvT] rows of chunk c.
    """
    xt = x.transpose(0, 2, 1).astype(np.float16)           # [2, 512, 2048]
    xtc = [np.ascontiguousarray(
        xt[bb].reshape(NCH, P, S).transpose(1, 0, 2).reshape(P, NCH * S))
        for bb in range(B)]
    w = Wqkv.reshape(3, D, D)
    in_maps = []
    for core in range(8):
        bb = core // 4
        g = core % 4
        r = slice(P * g, P * (g + 1))
        wt = np.concatenate([w[0][r].T, w[1][r].T, w[2][r].T],
                            axis=1).astype(np.float16)      # [512, 384]
        wtc = wt.reshape(NCH, P, 3 * P).transpose(1, 0, 2).reshape(P, NCH * 3 * P)
        in_maps.append({"xt": xtc[bb], "wt": np.ascontiguousarray(wtc)})
    return in_maps


def kernel(x, Wqkv, Wo, scale_q, scale_k, mask):
    x = np.asarray(x, np.float32)
    Wqkv = np.asarray(Wqkv, np.float32)
    Wo = np.asarray(Wo, np.float32)
    causal = np.triu(np.ones((S, S), dtype=bool), k=1)
    if (not np.allclose(np.asarray(scale_q), 1.0) or
            not np.allclose(np.asarray(scale_k), 1.0) or
            not np.array_equal(np.asarray(mask), causal) or
            x.shape != (B, S, D)):
        return _numpy_reference(x, Wqkv, Wo, np.asarray(scale_q), np.asarray(scale_k),
                                np.asarray(mask))

    runner = _get_runner()
    in_maps = _prep_inputs(x, Wqkv)
    _CACHE["last_in_maps"] = in_maps
    concat_in = runner.concat_inputs(in_maps)
    outs = runner.run_concat(concat_in)
    res = runner.fetch(outs)
    # host epilogue: out[b] = concat_g(z_g) @ Wo^T
    outp = np.empty((B, S, D), np.float32)
    WoT = np.ascontiguousarray(Wo.T)
    for bb in range(B):
        zb = np.concatenate([res[4 * bb + g]["z"] for g in range(4)],
                            axis=1).astype(np.float32)     # [2048, 512]
        np.dot(zb, WoT, out=outp[bb])
    return outp


# revision 35
# speedup vs baseline: 3324.7776x; 24.3187x over previous
"""Trainium2 Bass kernel for nn_CausalVideoAttention (b=2, s=2048, d=512, 8 heads).

Sharding: 8 cores = (batch, head-pair): core c -> batch c//4, heads {2*(c%4), 2*(c%4)+1}.
Each core computes the qkv projection for its head pair (fp16 inputs, f32 psum),
rms-normalizes q and k (factors folded in on-chip), runs causal attention over the
full sequence, and returns normalized per-pair z in native [seq, dh*2] orientation
(fp16). The host applies the output projection Wo (one sgemm per batch) and
concatenates head pairs. No device collectives.

On-chip layout: activations stored [feature, seq] ("transposed domain") so every
projection/score matmul contracts over partitions; the z accumulation is done with
q on partitions (out free = 65 = dh+denominator) which halves PE row count vs the
wide-free form and makes softmax normalization a per-partition scalar multiply.
"""

import sys

for _p in ("/opt/trn_rl_repo",):
    if _p not in sys.path:
        sys.path.insert(0, _p)

import numpy as np

B, S, D = 2, 2048, 512
NH, DH = 8, 64
P = 128          # partitions / tile edge
NCH = D // P     # 4 din chunks
QC = 512         # q-chunk width
NQC = S // QC    # 4 q-chunks
NT = S // P      # 16 s-tiles
EPS = 1e-6

_CACHE = {}


def _build_program(debug_taps=False):
    import concourse.bass as bass  # noqa: F401
    import concourse.mybir as mybir
    import concourse.tile as tile
    from concourse import bacc

    F32 = mybir.dt.float32
    F32R = mybir.dt.float32r
    F16 = mybir.dt.float16
    BF16 = mybir.dt.bfloat16
    AF = mybir.ActivationFunctionType

    nc = bacc.Bacc("TRN2", target_bir_lowering=False, debug=False, num_devices=8)
    xt = nc.dram_tensor("xt", [P, NCH * S], F16, kind="ExternalInput").ap()
    wt = nc.dram_tensor("wt", [P, NCH * 3 * P], F16, kind="ExternalInput").ap()
    z = nc.dram_tensor("z", [S, P], F16, kind="ExternalOutput").ap()
    taps = {}
    if debug_taps:
        taps["dq"] = nc.dram_tensor("dq", [P, S], F16, kind="ExternalOutput").ap()
        taps["dk"] = nc.dram_tensor("dk", [P, S], F16, kind="ExternalOutput").ap()
        taps["dv0"] = nc.dram_tensor("dv0", [P, 66 * NT], F32, kind="ExternalOutput").ap()
        taps["dv1"] = nc.dram_tensor("dv1", [P, 66 * NT], F32, kind="ExternalOutput").ap()
        for kk in range(4):
            taps[f"de{kk}"] = nc.dram_tensor(f"de{kk}", [P, 1024], F32, kind="ExternalOutput").ap()
        taps["dzt"] = nc.dram_tensor("dzt", [P, 264], F32, kind="ExternalOutput").ap()

    with tile.TileContext(nc) as tc:
        with tc.tile_pool(name="const", bufs=1) as cpool, \
             tc.tile_pool(name="big", bufs=1) as big:
            # selector [128, 2]: col h sums partitions of head h (for sum-of-squares)
            sel2 = cpool.tile([P, 2], F16, tag="sel2")
            nc.vector.memset(sel2[:], 0.0)
            nc.vector.memset(sel2[0:64, 0:1], 1.0)
            nc.vector.memset(sel2[64:128, 1:2], 1.0)
            # broadcast selector [2, 128]: bsel[p, i] = 1 iff i // 64 == p,
            # built as ones gated by two affine selects (64p <= i < 64p+64)
            bsel = cpool.tile([2, P], F32R, tag="bsel")
            bselt = cpool.tile([2, P], F32, tag="bselt")
            nc.vector.memset(bselt[:], 1.0)
            nc.gpsimd.affine_select(out=bselt[:], in_=bselt[:],
                                    compare_op=mybir.AluOpType.is_ge, fill=0.0,
                                    base=0, channel_multiplier=-64, pattern=[[1, P]])
            nc.gpsimd.affine_select(out=bselt[:], in_=bselt[:],
                                    compare_op=mybir.AluOpType.is_ge, fill=0.0,
                                    base=63, channel_multiplier=64, pattern=[[-1, P]])
            nc.gpsimd.tensor_copy(bsel[:], bselt[:])

            # persistent tiles
            xts = big.tile([P, NCH * S], F16, tag="xts")        # x^T din-chunks
            wts = big.tile([P, NCH * 3 * P], F16, tag="wts")    # w^T din-chunks
            qTn = big.tile([P, S], F16, tag="qTn")              # normalized q^T
            kTn = big.tile([P, S], F16, tag="kTn")              # normalized k^T
            # v augmented with a ones column per s-tile: [v_h (64) | 1]
            vaug = [big.tile([P, 66 * NT], BF16, name=f"vaug{h}", tag=f"vaug{h}")
                    for h in range(2)]

            for h in range(2):
                nc.vector.memset(vaug[h][:], 1.0)
            # one plain 2D DMA per input (host supplies chunk-major layout):
            # a single queue/completion event per tensor keeps every
            # consumer's wait identical, so nothing dispatches out of order
            nc.sync.dma_start(xts[:], xt[:])
            nc.sync.dma_start(wts[:], wt[:])

            # ============ phase 1: projections + rmsnorm factors ============
            with tc.tile_pool(name="pps", bufs=2, space="PSUM") as pps, \
                 tc.tile_pool(name="sps", bufs=1, space="PSUM") as sps, \
                 tc.tile_pool(name="fps", bufs=1, space="PSUM") as fps, \
                 tc.tile_pool(name="vps", bufs=2, space="PSUM") as vps, \
                 tc.tile_pool(name="sq", bufs=2) as sqp, \
                 tc.tile_pool(name="fr", bufs=2) as frp:
                for sc in range(NQC):
                    qs = slice(QC * sc, QC * (sc + 1))
                    ssqk = sps.tile([2, 2 * QC], F32, tag="ssqk")
                    sq = sqp.tile([P, 2 * QC], F16, tag="sq")
                    raws = []
                    for ki, w0 in ((0, 0), (1, P)):  # q then k
                        ps = pps.tile([P, QC], F32, tag="proj")
                        for c in range(NCH):
                            nc.tensor.matmul(
                                ps[:],
                                wts[:, 3 * P * c + w0:3 * P * c + w0 + P],
                                xts[:, S * c + QC * sc:S * c + QC * (sc + 1)],
                                start=(c == 0), stop=(c == 3))
                        raw = sqp.tile([P, QC], F16, tag="raw")
                        nc.vector.tensor_copy(raw[:], ps[:])
                        nc.gpsimd.tensor_mul(sq[:, QC * ki:QC * (ki + 1)],
                                             raw[:], raw[:])
                        nc.tensor.matmul(ssqk[:, QC * ki:QC * (ki + 1)], sel2[:],
                                         sq[:, QC * ki:QC * (ki + 1)],
                                         start=True, stop=True)
                        raws.append(raw)
                    # factors: 1/sqrt(mean sq); rows = heads, cols = [fq | fk]
                    srt = frp.tile([2, 2 * QC], F32, tag="srt")
                    ftmp = frp.tile([2, 2 * QC], F32, tag="ftmp")
                    frec = frp.tile([2, 2 * QC], F32R, tag="frec")
                    nc.scalar.activation(srt[:], ssqk[:], AF.Sqrt,
                                         bias=0.0, scale=1.0 / DH)
                    nc.vector.reciprocal_approx_fast(ftmp[:], srt[:])
                    nc.gpsimd.tensor_copy(frec[:], ftmp[:])
                    # broadcast factor rows across dh partitions via PE
                    for ki, dst in ((0, qTn), (1, kTn)):
                        fb = fps.tile([P, QC], F32, tag="fb")
                        nc.tensor.matmul(fb[:], bsel[:],
                                         frec[:, QC * ki:QC * (ki + 1)],
                                         start=True, stop=True)
                        nc.vector.tensor_mul(dst[:, qs], raws[ki][:], fb[:])
                    # v in native orientation [s-pos, dh] with ones column
                    psv = vps.tile([P, QC], F32, tag="psv")
                    for tl in range(4):
                        t = 4 * sc + tl
                        for c in range(NCH):
                            nc.tensor.matmul(
                                psv[:, P * tl:P * (tl + 1)],
                                xts[:, S * c + P * t:S * c + P * (t + 1)],
                                wts[:, 3 * P * c + 2 * P:3 * P * (c + 1)],
                                start=(c == 0), stop=(c == 3))
                    for h in range(2):
                        for tl in range(4):
                            t = 4 * sc + tl
                            nc.vector.tensor_copy(
                                vaug[h][:, 66 * t:66 * t + 64],
                                psv[:, P * tl + 64 * h:P * tl + 64 * (h + 1)])

            # ================= phase 2: attention =================
            with tc.tile_pool(name="scps", bufs=2, space="PSUM") as scps, \
                 tc.tile_pool(name="ztps", bufs=2, space="PSUM") as ztps, \
                 tc.tile_pool(name="att", bufs=18) as att, \
                 tc.tile_pool(name="nrm", bufs=2) as nrm, \
                 tc.tile_pool(name="zo", bufs=2) as zop:
                for j in range(NQC):
                    q0 = QC * j
                    nkb = 4 * j + 4
                    zts = [ztps.tile([P, 264], F32, name=f"zt{j}_{h}", tag=f"zt{h}")
                           for h in range(2)]
                    eTs = []
                    for kb in range(nkb):
                        i = kb - 4 * j
                        qoff = max(0, P * i)
                        fr = QC - qoff
                        # head h occupies cols [512h : 512h + fr] so each
                        # matmul output stays inside one 512-float psum bank
                        sc_ps = scps.tile([P, 1024], F32, tag="sc")
                        eT = att.tile([P, 1024], BF16, tag="eT")
                        for h in range(2):
                            hs = slice(64 * h, 64 * (h + 1))
                            nc.tensor.matmul(sc_ps[:, QC * h:QC * h + fr],
                                             kTn[hs, P * kb:P * (kb + 1)],
                                             qTn[hs, q0 + qoff:q0 + QC],
                                             start=True, stop=True)
                        both = slice(0, QC + fr) if fr == QC else None
                        if both is not None:
                            nc.scalar.activation(eT[:, both], sc_ps[:, both],
                                                 AF.Exp, bias=0.0, scale=1.0)
                        else:
                            ap_o = eT[:].rearrange("p (h f) -> p h f", h=2)[:, :, 0:fr]
                            ap_i = sc_ps[:].rearrange("p (h f) -> p h f", h=2)[:, :, 0:fr]
                            nc.scalar.activation(ap_o, ap_i,
                                                 AF.Exp, bias=0.0, scale=1.0)
                        if i >= 0:
                            # zero the strictly-upper triangle of the diagonal
                            # 128-col block of each head (keep where col >= p)
                            tri = eT[:].rearrange("p (h f) -> p h f", h=2)[:, :, 0:P]
                            nc.gpsimd.affine_select(
                                out=tri, in_=tri,
                                compare_op=mybir.AluOpType.is_ge, fill=0.0,
                                base=0, channel_multiplier=-1,
                                pattern=[[0, 2], [1, P]])
                        if debug_taps and j == 0:
                            pass  # de taps disabled for bf16 eT
                        eTs.append((kb, qoff, fr, eT))
                    # z accumulation: q on partitions, free = [v 64 | denom].
                    # qsub-major so each psum region's accumulation group
                    # closes before the next one starts (a psum bank holds
                    # only one open accumulation group at a time).
                    for qsub in range(4):
                        for h in range(2):
                            for kb, qoff, fr, eT in eTs[:4 * j + qsub + 1]:
                                if P * qsub < qoff:
                                    continue
                                nc.tensor.matmul(
                                    zts[h][:, 66 * qsub:66 * (qsub + 1)],
                                    eT[:, QC * h + P * qsub - qoff:
                                          QC * h + P * qsub - qoff + P],
                                    vaug[h][:, 66 * kb:66 * (kb + 1)],
                                    start=(kb == 0), stop=(kb == 4 * j + qsub))
                    zsb = zop.tile([P, 4 * P], F16, tag="zsb")
                    for h in range(2):
                        rcp = nrm.tile([P, 4], F32, tag="rcp")
                        nc.vector.reciprocal(rcp[:], zts[h][:, 64::66])
                        for qsub in range(4):
                            nc.vector.tensor_scalar(
                                out=zsb[:, P * qsub + 64 * h:P * qsub + 64 * (h + 1)],
                                in0=zts[h][:, 66 * qsub:66 * qsub + 64],
                                scalar1=rcp[:, qsub:qsub + 1],
                                scalar2=None,
                                op0=mybir.AluOpType.mult)
                    nc.sync.dma_start(
                        z[q0:q0 + QC, :].rearrange("(qs p) c -> p qs c", p=P),
                        zsb[:].rearrange("p (qs c) -> p qs c", qs=4))
                    if debug_taps and j == 0:
                        zcopy = zop.tile([P, 264], F32, tag="zcopy")
                        nc.vector.tensor_copy(zcopy[:], zts[0][:])
                        nc.sync.dma_start(taps["dzt"], zcopy[:])
            if debug_taps:
                nc.sync.dma_start(taps["dq"], qTn[:])
                nc.sync.dma_start(taps["dk"], kTn[:])
                pass
                pass

    nc.finalize()
    return nc


def _numpy_reference(x, Wqkv, Wo, scale_q, scale_k, mask):
    b, s, d = x.shape
    dh = d // NH
    qkv = x @ Wqkv.T
    q, k, v = np.split(qkv, 3, axis=-1)

    def rms(t, scale):
        r = np.sqrt(np.mean(np.square(t), axis=-1, keepdims=True)) + EPS
        return t / r * scale

    q = rms(q.reshape(b, s, NH, dh), scale_q)
    k = rms(k.reshape(b, s, NH, dh), scale_k)
    v = v.reshape(b, s, NH, dh)
    attn = np.einsum('bqhd,bkhd->bhqk', q, k)
    attn = np.where(mask[None, None], -np.inf, attn)
    attn = attn - attn.max(axis=-1, keepdims=True)
    p = np.exp(attn)
    p = p / p.sum(axis=-1, keepdims=True)
    zz = np.einsum('bhqk,bkhd->bqhd', p, v).reshape(b, s, d)
    return (zz @ Wo.T).astype(np.float32)


class _Runner:
    """Caches the jitted shard_map executable for nc across calls."""

    def __init__(self, nc, n_cores=8):
        import jax
        import concourse.mybir as mybir
        from jax.sharding import Mesh, PartitionSpec
        from jax.experimental.shard_map import shard_map
        from concourse.bass2jax import (
            _bass_exec_p, install_neuronx_cc_hook, partition_id_tensor)

        install_neuronx_cc_hook()
        self.nc = nc
        self.n_cores = n_cores
        partition_name = nc.partition_id_tensor.name if nc.partition_id_tensor else None
        in_names, out_names, out_avals, zero_shapes = [], [], [], []
        for alloc in nc.m.functions[0].allocations:
            if not isinstance(alloc, mybir.MemoryLocationSet):
                continue
            name = alloc.memorylocations[0].name
            if alloc.kind == "ExternalInput":
                if name != partition_name:
                    in_names.append(name)
            elif alloc.kind == "ExternalOutput":
                out_names.append(name)
                shape = tuple(alloc.tensor_shape)
                dtype = mybir.dt.np(alloc.dtype)
                out_avals.append(jax.core.ShapedArray(shape, dtype))
                zero_shapes.append((shape, dtype))
        self.in_names = in_names
        self.out_names = out_names
        self.zero_shapes = zero_shapes
        n_params = len(in_names)
        n_outs = len(out_avals)
        in_names_all = in_names + out_names + ([partition_name] if partition_name else [])
        donate = tuple(range(n_params, n_params + n_outs))

        def _body(*args):
            operands = list(args)
            if partition_name is not None:
                operands.append(partition_id_tensor())
            outs = _bass_exec_p.bind(
                *operands, out_avals=tuple(out_avals),
                in_names=tuple(in_names_all), out_names=tuple(out_names),
                lowering_input_output_aliases=(),
                sim_require_finite=True, sim_require_nnan=True, nc=nc)
            return tuple(outs)

        devices = jax.devices()[:n_cores]
        self.mesh = Mesh(np.asarray(devices), ("core",))
        in_specs = (PartitionSpec("core"),) * (n_params + n_outs)
        out_specs = (PartitionSpec("core"),) * n_outs
        self.sharded = jax.jit(
            shard_map(_body, mesh=self.mesh, in_specs=in_specs,
                      out_specs=out_specs, check_rep=False),
            donate_argnums=donate, keep_unused=True)
        self._zmaker = jax.jit(
            lambda: tuple(
                jax.numpy.zeros((n_cores * sh[0], *sh[1:]), dt)
                for sh, dt in zero_shapes),
            out_shardings=tuple(
                jax.sharding.NamedSharding(self.mesh, PartitionSpec("core"))
                for _ in zero_shapes))

    def concat_inputs(self, in_maps):
        return [np.concatenate([np.asarray(m[nm]) for m in in_maps], axis=0)
                for nm in self.in_names]

    def run_concat(self, concat_in):
        """concat_in: list of (n_cores*dim0, ...) arrays (np or device)."""
        zeros = self._zmaker()
        outs = self.sharded(*concat_in, *zeros)
        return outs

    def fetch(self, outs):
        res = []
        for c in range(self.n_cores):
            d = {}
            for i, name in enumerate(self.out_names):
                sh = self.zero_shapes[i][0]
                d[name] = np.asarray(outs[i]).reshape(self.n_cores, *sh)[c]
            res.append(d)
        return res


def _get_runner():
    if "runner" not in _CACHE:
        _CACHE["runner"] = _Runner(_build_program())
    return _CACHE["runner"]


def _prep_inputs(x, Wqkv):
    """Build per-core device inputs: fp16, transposed, din-chunk-major.

    xt: [128, 4*2048] where chunk c cols hold x[b].T rows 128c:128c+128.
    wt: [128, 4*384]  where chunk c holds [WqT | WkT | WvT] rows of chunk c.
    """
    xt = x.transpose(0, 2, 1).astype(np.float16)           # [2, 512, 2048]
    xtc = [np.ascontiguousarray(
        xt[bb].reshape(NCH, P, S).transpose(1, 0, 2).reshape(P, NCH * S))
        for bb in range(B)]
    w = Wqkv.reshape(3, D, D)
    in_maps = []
    for core in range(8):
        bb = core // 4
        g = core % 4
        r = slice(P * g, P * (g + 1))
        wt = np.concatenate([w[0][r].T, w[1][r].T, w[2][r].T],
                            axis=1).astype(np.float16)      # [512, 384]
        wtc = wt.reshape(NCH, P, 3 * P).transpose(1, 0, 2).reshape(P, NCH * 3 * P)
        in_maps.append({"xt": xtc[bb], "wt": np.ascontiguousarray(wtc)})
    return in_maps


def kernel(x, Wqkv, Wo, scale_q, scale_k, mask):
    x = np.asarray(x, np.float32)
    Wqkv = np.asarray(Wqkv, np.float32)
    Wo = np.asarray(Wo, np.float32)
    causal = np.triu(np.ones((S, S), dtype=bool), k=1)
    if (not np.allclose(np.asarray(scale_q), 1.0) or
            not np.allclose(np.asarray(scale_k), 1.0) or
            not np.array_equal(np.asarray(mask), causal) or
            x.shape != (B, S, D)):
        return _numpy_reference(x, Wqkv, Wo, np.asarray(scale_q), np.asarray(scale_k),
                                np.asarray(mask))

    runner = _get_runner()
    in_maps = _prep_inputs(x, Wqkv)
    _CACHE["last_in_maps"] = in_maps
    concat_in = runner.concat_inputs(in_maps)
    outs = runner.run_concat(concat_in)
    res = runner.fetch(outs)
    # host epilogue: out[b] = concat_g(z_g) @ Wo^T
    outp = np.empty((B, S, D), np.float32)
    WoT = np.ascontiguousarray(Wo.T)
    for bb in range(B):
        zb = np.concatenate([res[4 * bb + g]["z"] for g in range(4)],
                            axis=1).astype(np.float32)     # [2048, 512]
        np.dot(zb, WoT, out=outp[bb])
    return outp


# revision 39
# speedup vs baseline: 3454.4767x; 1.0390x over previous
"""Trainium2 Bass kernel for nn_CausalVideoAttention (b=2, s=2048, d=512, 8 heads).

Sharding: 8 cores = (batch, head-pair): core c -> batch c//4, heads {2*(c%4), 2*(c%4)+1}.
Each core computes the qkv projection for its head pair (fp16 inputs, f32 psum),
rms-normalizes q and k (factors folded in on-chip), runs causal attention over the
full sequence, and returns normalized per-pair z in native [seq, dh*2] orientation
(fp16). The host applies the output projection Wo (one sgemm per batch) and
concatenates head pairs. No device collectives.

On-chip layout: activations stored [feature, seq] ("transposed domain") so every
projection/score matmul contracts over partitions; the z accumulation is done with
q on partitions (out free = 65 = dh+denominator) which halves PE row count vs the
wide-free form and makes softmax normalization a per-partition scalar multiply.
"""

import sys

for _p in ("/opt/trn_rl_repo",):
    if _p not in sys.path:
        sys.path.insert(0, _p)

import numpy as np

B, S, D = 2, 2048, 512
NH, DH = 8, 64
P = 128          # partitions / tile edge
NCH = D // P     # 4 din chunks
QC = 512         # q-chunk width
NQC = S // QC    # 4 q-chunks
NT = S // P      # 16 s-tiles
EPS = 1e-6

_CACHE = {}


def _build_program(debug_taps=False):
    import concourse.bass as bass  # noqa: F401
    import concourse.mybir as mybir
    import concourse.tile as tile
    from concourse import bacc

    F32 = mybir.dt.float32
    F32R = mybir.dt.float32r
    F16 = mybir.dt.float16
    BF16 = mybir.dt.bfloat16
    AF = mybir.ActivationFunctionType

    nc = bacc.Bacc("TRN2", target_bir_lowering=False, debug=False, num_devices=8)
    xt = nc.dram_tensor("xt", [P, NCH * S], F16, kind="ExternalInput").ap()
    wt = nc.dram_tensor("wt", [P, NCH * 3 * P], F16, kind="ExternalInput").ap()
    z = nc.dram_tensor("z", [S, P], F16, kind="ExternalOutput").ap()
    taps = {}
    if debug_taps:
        taps["dq"] = nc.dram_tensor("dq", [P, S], F16, kind="ExternalOutput").ap()
        taps["dk"] = nc.dram_tensor("dk", [P, S], F16, kind="ExternalOutput").ap()
        taps["dv0"] = nc.dram_tensor("dv0", [P, 66 * NT], F32, kind="ExternalOutput").ap()
        taps["dv1"] = nc.dram_tensor("dv1", [P, 66 * NT], F32, kind="ExternalOutput").ap()
        for kk in range(4):
            taps[f"de{kk}"] = nc.dram_tensor(f"de{kk}", [P, 1024], F32, kind="ExternalOutput").ap()
        taps["dzt"] = nc.dram_tensor("dzt", [P, 264], F32, kind="ExternalOutput").ap()

    with tile.TileContext(nc) as tc:
        with tc.tile_pool(name="const", bufs=1) as cpool, \
             tc.tile_pool(name="big", bufs=1) as big:
            # selector [128, 2]: col h sums partitions of head h (for sum-of-squares)
            sel2 = cpool.tile([P, 2], F16, tag="sel2")
            nc.vector.memset(sel2[:], 0.0)
            nc.vector.memset(sel2[0:64, 0:1], 1.0)
            nc.vector.memset(sel2[64:128, 1:2], 1.0)
            # broadcast selector [2, 128]: bsel[p, i] = 1 iff i // 64 == p,
            # built as ones gated by two affine selects (64p <= i < 64p+64)
            bsel = cpool.tile([2, P], F32R, tag="bsel")
            bselt = cpool.tile([2, P], F32, tag="bselt")
            nc.vector.memset(bselt[:], 1.0)
            nc.gpsimd.affine_select(out=bselt[:], in_=bselt[:],
                                    compare_op=mybir.AluOpType.is_ge, fill=0.0,
                                    base=0, channel_multiplier=-64, pattern=[[1, P]])
            nc.gpsimd.affine_select(out=bselt[:], in_=bselt[:],
                                    compare_op=mybir.AluOpType.is_ge, fill=0.0,
                                    base=63, channel_multiplier=64, pattern=[[-1, P]])
            nc.gpsimd.tensor_copy(bsel[:], bselt[:])
            # act-table prefetch: one dummy exp then sqrt so the table loads
            # overlap the input DMA instead of the critical path
            warm = cpool.tile([1, 2], F32, tag="warm")
            nc.vector.memset(warm[:], 1.0)
            nc.scalar.activation(warm[:, 0:1], warm[:, 0:1], AF.Exp,
                                 bias=0.0, scale=1.0)
            nc.scalar.activation(warm[:, 1:2], warm[:, 1:2], AF.Sqrt,
                                 bias=0.0, scale=1.0)

            # persistent tiles
            xts = big.tile([P, NCH * S], F16, tag="xts")        # x^T din-chunks
            wts = big.tile([P, NCH * 3 * P], F16, tag="wts")    # w^T din-chunks
            qTn = big.tile([P, S], F16, tag="qTn")              # normalized q^T
            kTn = big.tile([P, S], F16, tag="kTn")              # normalized k^T
            # v augmented with a ones column per s-tile: [v_h (64) | 1]
            vaug = [big.tile([P, 66 * NT], BF16, name=f"vaug{h}", tag=f"vaug{h}")
                    for h in range(2)]

            for h in range(2):
                nc.vector.memset(vaug[h][:], 1.0)
            # per-q-chunk input DMAs (each delivers that chunk's columns of
            # every din chunk, so all its consumers share one completion
            # event); first projection starts after 1/4 of x has landed
            nc.sync.dma_start(wts[:], wt[:])
            for sc in range(NQC):
                cols = [slice(None), slice(None), slice(QC * sc, QC * (sc + 1))]
                nc.sync.dma_start(
                    xts[:].rearrange("p (c s) -> p c s", c=NCH)[tuple(cols)],
                    xt.rearrange("p (c s) -> p c s", c=NCH)[tuple(cols)])

            # ============ phase 1: projections + rmsnorm factors ============
            with tc.tile_pool(name="pps", bufs=2, space="PSUM") as pps, \
                 tc.tile_pool(name="sps", bufs=1, space="PSUM") as sps, \
                 tc.tile_pool(name="fps", bufs=1, space="PSUM") as fps, \
                 tc.tile_pool(name="vps", bufs=2, space="PSUM") as vps, \
                 tc.tile_pool(name="sq", bufs=2) as sqp, \
                 tc.tile_pool(name="fr", bufs=2) as frp:
                for sc in range(NQC):
                    qs = slice(QC * sc, QC * (sc + 1))
                    ssqk = sps.tile([2, 2 * QC], F32, tag="ssqk")
                    sq = sqp.tile([P, 2 * QC], F16, tag="sq")
                    raws = []
                    for ki, w0 in ((0, 0), (1, P)):  # q then k
                        ps = pps.tile([P, QC], F32, tag="proj")
                        for c in range(NCH):
                            nc.tensor.matmul(
                                ps[:],
                                wts[:, 3 * P * c + w0:3 * P * c + w0 + P],
                                xts[:, S * c + QC * sc:S * c + QC * (sc + 1)],
                                start=(c == 0), stop=(c == 3))
                        raw = sqp.tile([P, QC], F16, tag="raw")
                        nc.vector.tensor_copy(raw[:], ps[:])
                        nc.gpsimd.tensor_mul(sq[:, QC * ki:QC * (ki + 1)],
                                             raw[:], raw[:])
                        nc.tensor.matmul(ssqk[:, QC * ki:QC * (ki + 1)], sel2[:],
                                         sq[:, QC * ki:QC * (ki + 1)],
                                         start=True, stop=True)
                        raws.append(raw)
                    # factors: 1/sqrt(mean sq); rows = heads, cols = [fq | fk]
                    srt = frp.tile([2, 2 * QC], F32, tag="srt")
                    ftmp = frp.tile([2, 2 * QC], F32, tag="ftmp")
                    frec = frp.tile([2, 2 * QC], F32R, tag="frec")
                    nc.scalar.activation(srt[:], ssqk[:], AF.Sqrt,
                                         bias=0.0, scale=1.0 / DH)
                    nc.vector.reciprocal_approx_fast(ftmp[:], srt[:])
                    nc.gpsimd.tensor_copy(frec[:], ftmp[:])
                    # broadcast factor rows across dh partitions via PE
                    for ki, dst in ((0, qTn), (1, kTn)):
                        fb = fps.tile([P, QC], F32, tag="fb")
                        nc.tensor.matmul(fb[:], bsel[:],
                                         frec[:, QC * ki:QC * (ki + 1)],
                                         start=True, stop=True)
                        nc.vector.tensor_mul(dst[:, qs], raws[ki][:], fb[:])
                    # v in native orientation [s-pos, dh] with ones column
                    psv = vps.tile([P, QC], F32, tag="psv")
                    for tl in range(4):
                        t = 4 * sc + tl
                        for c in range(NCH):
                            nc.tensor.matmul(
                                psv[:, P * tl:P * (tl + 1)],
                                xts[:, S * c + P * t:S * c + P * (t + 1)],
                                wts[:, 3 * P * c + 2 * P:3 * P * (c + 1)],
                                start=(c == 0), stop=(c == 3))
                    for h in range(2):
                        for tl in range(4):
                            t = 4 * sc + tl
                            nc.vector.tensor_copy(
                                vaug[h][:, 66 * t:66 * t + 64],
                                psv[:, P * tl + 64 * h:P * tl + 64 * (h + 1)])

            # ================= phase 2: attention =================
            with tc.tile_pool(name="scps", bufs=2, space="PSUM") as scps, \
                 tc.tile_pool(name="ztps", bufs=2, space="PSUM") as ztps, \
                 tc.tile_pool(name="att", bufs=18) as att, \
                 tc.tile_pool(name="nrm", bufs=2) as nrm, \
                 tc.tile_pool(name="zo", bufs=2) as zop:
                for j in range(NQC):
                    q0 = QC * j
                    nkb = 4 * j + 4
                    zts = [ztps.tile([P, 264], F32, name=f"zt{j}_{h}", tag=f"zt{h}")
                           for h in range(2)]
                    eTs = []
                    for kb in range(nkb):
                        i = kb - 4 * j
                        qoff = max(0, P * i)
                        fr = QC - qoff
                        # head h occupies cols [512h : 512h + fr] so each
                        # matmul output stays inside one 512-float psum bank
                        sc_ps = scps.tile([P, 1024], F32, tag="sc")
                        eT = att.tile([P, 1024], BF16, tag="eT")
                        for h in range(2):
                            hs = slice(64 * h, 64 * (h + 1))
                            nc.tensor.matmul(sc_ps[:, QC * h:QC * h + fr],
                                             kTn[hs, P * kb:P * (kb + 1)],
                                             qTn[hs, q0 + qoff:q0 + QC],
                                             start=True, stop=True)
                        both = slice(0, QC + fr) if fr == QC else None
                        if both is not None:
                            nc.scalar.activation(eT[:, both], sc_ps[:, both],
                                                 AF.Exp, bias=0.0, scale=1.0)
                        else:
                            ap_o = eT[:].rearrange("p (h f) -> p h f", h=2)[:, :, 0:fr]
                            ap_i = sc_ps[:].rearrange("p (h f) -> p h f", h=2)[:, :, 0:fr]
                            nc.scalar.activation(ap_o, ap_i,
                                                 AF.Exp, bias=0.0, scale=1.0)
                        if i >= 0:
                            # zero the strictly-upper triangle of the diagonal
                            # 128-col block of each head (keep where col >= p)
                            tri = eT[:].rearrange("p (h f) -> p h f", h=2)[:, :, 0:P]
                            nc.gpsimd.affine_select(
                                out=tri, in_=tri,
                                compare_op=mybir.AluOpType.is_ge, fill=0.0,
                                base=0, channel_multiplier=-1,
                                pattern=[[0, 2], [1, P]])
                        if debug_taps and j == 0:
                            pass  # de taps disabled for bf16 eT
                        eTs.append((kb, qoff, fr, eT))
                    # z accumulation: q on partitions, free = [v 64 | denom].
                    # qsub-major so each psum region's accumulation group
                    # closes before the next one starts (a psum bank holds
                    # only one open accumulation group at a time).
                    for qsub in range(4):
                        for h in range(2):
                            for kb, qoff, fr, eT in eTs[:4 * j + qsub + 1]:
                                if P * qsub < qoff:
                                    continue
                                nc.tensor.matmul(
                                    zts[h][:, 66 * qsub:66 * (qsub + 1)],
                                    eT[:, QC * h + P * qsub - qoff:
                                          QC * h + P * qsub - qoff + P],
                                    vaug[h][:, 66 * kb:66 * (kb + 1)],
                                    start=(kb == 0), stop=(kb == 4 * j + qsub))
                    zsb = zop.tile([P, 4 * P], F16, tag="zsb")
                    for h in range(2):
                        rcp = nrm.tile([P, 4], F32, tag="rcp")
                        nc.vector.reciprocal(rcp[:], zts[h][:, 64::66])
                        for qsub in range(4):
                            nc.vector.tensor_scalar(
                                out=zsb[:, P * qsub + 64 * h:P * qsub + 64 * (h + 1)],
                                in0=zts[h][:, 66 * qsub:66 * qsub + 64],
                                scalar1=rcp[:, qsub:qsub + 1],
                                scalar2=None,
                                op0=mybir.AluOpType.mult)
                    nc.sync.dma_start(
                        z[q0:q0 + QC, :].rearrange("(qs p) c -> p qs c", p=P),
                        zsb[:].rearrange("p (qs c) -> p qs c", qs=4))
                    if debug_taps and j == 0:
                        zcopy = zop.tile([P, 264], F32, tag="zcopy")
                        nc.vector.tensor_copy(zcopy[:], zts[0][:])
                        nc.sync.dma_start(taps["dzt"], zcopy[:])
            if debug_taps:
                nc.sync.dma_start(taps["dq"], qTn[:])
                nc.sync.dma_start(taps["dk"], kTn[:])
                pass
                pass

    nc.finalize()
    return nc


def _numpy_reference(x, Wqkv, Wo, scale_q, scale_k, mask):
    b, s, d = x.shape
    dh = d // NH
    qkv = x @ Wqkv.T
    q, k, v = np.split(qkv, 3, axis=-1)

    def rms(t, scale):
        r = np.sqrt(np.mean(np.square(t), axis=-1, keepdims=True)) + EPS
        return t / r * scale

    q = rms(q.reshape(b, s, NH, dh), scale_q)
    k = rms(k.reshape(b, s, NH, dh), scale_k)
    v = v.reshape(b, s, NH, dh)
    attn = np.einsum('bqhd,bkhd->bhqk', q, k)
    attn = np.where(mask[None, None], -np.inf, attn)
    attn = attn - attn.max(axis=-1, keepdims=True)
    p = np.exp(attn)
    p = p / p.sum(axis=-1, keepdims=True)
    zz = np.einsum('bhqk,bkhd->bqhd', p, v).reshape(b, s, d)
    return (zz @ Wo.T).astype(np.float32)


class _Runner:
    """Caches the jitted shard_map executable for nc across calls."""

    def __init__(self, nc, n_cores=8):
        import jax
        import concourse.mybir as mybir
        from jax.sharding import Mesh, PartitionSpec
        from jax.experimental.shard_map import shard_map
        from concourse.bass2jax import (
            _bass_exec_p, install_neuronx_cc_hook, partition_id_tensor)

        install_neuronx_cc_hook()
        self.nc = nc
        self.n_cores = n_cores
        partition_name = nc.partition_id_tensor.name if nc.partition_id_tensor else None
        in_names, out_names, out_avals, zero_shapes = [], [], [], []
        for alloc in nc.m.functions[0].allocations:
            if not isinstance(alloc, mybir.MemoryLocationSet):
                continue
            name = alloc.memorylocations[0].name
            if alloc.kind == "ExternalInput":
                if name != partition_name:
                    in_names.append(name)
            elif alloc.kind == "ExternalOutput":
                out_names.append(name)
                shape = tuple(alloc.tensor_shape)
                dtype = mybir.dt.np(alloc.dtype)
                out_avals.append(jax.core.ShapedArray(shape, dtype))
                zero_shapes.append((shape, dtype))
        self.in_names = in_names
        self.out_names = out_names
        self.zero_shapes = zero_shapes
        n_params = len(in_names)
        n_outs = len(out_avals)
        in_names_all = in_names + out_names + ([partition_name] if partition_name else [])
        donate = tuple(range(n_params, n_params + n_outs))

        def _body(*args):
            operands = list(args)
            if partition_name is not None:
                operands.append(partition_id_tensor())
            outs = _bass_exec_p.bind(
                *operands, out_avals=tuple(out_avals),
                in_names=tuple(in_names_all), out_names=tuple(out_names),
                lowering_input_output_aliases=(),
                sim_require_finite=True, sim_require_nnan=True, nc=nc)
            return tuple(outs)

        devices = jax.devices()[:n_cores]
        self.mesh = Mesh(np.asarray(devices), ("core",))
        in_specs = (PartitionSpec("core"),) * (n_params + n_outs)
        out_specs = (PartitionSpec("core"),) * n_outs
        self.sharded = jax.jit(
            shard_map(_body, mesh=self.mesh, in_specs=in_specs,
                      out_specs=out_specs, check_rep=False),
            donate_argnums=donate, keep_unused=True)
        self._zmaker = jax.jit(
            lambda: tuple(
                jax.numpy.zeros((n_cores * sh[0], *sh[1:]), dt)
                for sh, dt in zero_shapes),
            out_shardings=tuple(
                jax.sharding.NamedSharding(self.mesh, PartitionSpec("core"))
                for _ in zero_shapes))

    def concat_inputs(self, in_maps):
        return [np.concatenate([np.asarray(m[nm]) for m in in_maps], axis=0)
                for nm in self.in_names]

    def run_concat(self, concat_in):
        """concat_in: list of (n_cores*dim0, ...) arrays (np or device)."""
        zeros = self._zmaker()
        outs = self.sharded(*concat_in, *zeros)
        return outs

    def fetch(self, outs):
        res = []
        for c in range(self.n_cores):
            d = {}
            for i, name in enumerate(self.out_names):
                sh = self.zero_shapes[i][0]
                d[name] = np.asarray(outs[i]).reshape(self.n_cores, *sh)[c]
            res.append(d)
        return res


def _get_runner():
    if "runner" not in _CACHE:
        _CACHE["runner"] = _Runner(_build_program())
    return _CACHE["runner"]


def _prep_inputs(x, Wqkv):
    """Build per-core device inputs: fp16, transposed, din-chunk-major.

    xt: [128, 4*2048] where chunk c cols hold x[b].T rows 128c:128c+128.
    wt: [128, 4*384]  where chunk c holds [WqT | WkT | WvT] rows of chunk c.
    """
    xt = x.transpose(0, 2, 1).astype(np.float16)           # [2, 512, 2048]
    xtc = [np.ascontiguousarray(
        xt[bb].reshape(NCH, P, S).transpose(1, 0, 2).reshape(P, NCH * S))
        for bb in range(B)]
    w = Wqkv.reshape(3, D, D)
    in_maps = []
    for core in range(8):
        bb = core // 4
        g = core % 4
        r = slice(P * g, P * (g + 1))
        wt = np.concatenate([w[0][r].T, w[1][r].T, w[2][r].T],
                            axis=1).astype(np.float16)      # [512, 384]
        wtc = wt.reshape(NCH, P, 3 * P).transpose(1, 0, 2).reshape(P, NCH * 3 * P)
        in_maps.append({"xt": xtc[bb], "wt": np.ascontiguousarray(wtc)})
    return in_maps


def kernel(x, Wqkv, Wo, scale_q, scale_k, mask):
    x = np.asarray(x, np.float32)
    Wqkv = np.asarray(Wqkv, np.float32)
    Wo = np.asarray(Wo, np.float32)
    causal = np.triu(np.ones((S, S), dtype=bool), k=1)
    if (not np.allclose(np.asarray(scale_q), 1.0) or
            not np.allclose(np.asarray(scale_k), 1.0) or
            not np.array_equal(np.asarray(mask), causal) or
            x.shape != (B, S, D)):
        return _numpy_reference(x, Wqkv, Wo, np.asarray(scale_q), np.asarray(scale_k),
                                np.asarray(mask))

    runner = _get_runner()
    in_maps = _prep_inputs(x, Wqkv)
    _CACHE["last_in_maps"] = in_maps
    concat_in = runner.concat_inputs(in_maps)
    outs = runner.run_concat(concat_in)
    res = runner.fetch(outs)
    # host epilogue: out[b] = concat_g(z_g) @ Wo^T
    outp = np.empty((B, S, D), np.float32)
    WoT = np.ascontiguousarray(Wo.T)
    for bb in range(B):
        zb = np.concatenate([res[4 * bb + g]["z"] for g in range(4)],
                            axis=1).astype(np.float32)     # [2048, 512]
        np.dot(zb, WoT, out=outp[bb])
    return outp


# revision 41
# speedup vs baseline: 3522.5749x; 1.0197x over previous
"""Trainium2 Bass kernel for nn_CausalVideoAttention (b=2, s=2048, d=512, 8 heads).

Sharding: 8 cores = (batch, head-pair): core c -> batch c//4, heads {2*(c%4), 2*(c%4)+1}.
Each core computes the qkv projection for its head pair (fp16 inputs, f32 psum),
rms-normalizes q and k (factors folded in on-chip), runs causal attention over the
full sequence, and returns normalized per-pair z in native [seq, dh*2] orientation
(fp16). The host applies the output projection Wo (one sgemm per batch) and
concatenates head pairs. No device collectives.

On-chip layout: activations stored [feature, seq] ("transposed domain") so every
projection/score matmul contracts over partitions; the z accumulation is done with
q on partitions (out free = 65 = dh+denominator) which halves PE row count vs the
wide-free form and makes softmax normalization a per-partition scalar multiply.
"""

import sys

for _p in ("/opt/trn_rl_repo",):
    if _p not in sys.path:
        sys.path.insert(0, _p)

import numpy as np

B, S, D = 2, 2048, 512
NH, DH = 8, 64
P = 128          # partitions / tile edge
NCH = D // P     # 4 din chunks
QC = 512         # q-chunk width
NQC = S // QC    # 4 q-chunks
NT = S // P      # 16 s-tiles
EPS = 1e-6

_CACHE = {}


def _build_program(debug_taps=False):
    import concourse.bass as bass  # noqa: F401
    import concourse.mybir as mybir
    import concourse.tile as tile
    from concourse import bacc

    F32 = mybir.dt.float32
    F32R = mybir.dt.float32r
    F16 = mybir.dt.float16
    BF16 = mybir.dt.bfloat16
    AF = mybir.ActivationFunctionType

    nc = bacc.Bacc("TRN2", target_bir_lowering=False, debug=False, num_devices=8)
    xt = nc.dram_tensor("xt", [P, NCH * S], F16, kind="ExternalInput").ap()
    wt = nc.dram_tensor("wt", [P, NCH * 3 * P], F16, kind="ExternalInput").ap()
    z = nc.dram_tensor("z", [S, P], F16, kind="ExternalOutput").ap()
    taps = {}
    if debug_taps:
        taps["dq"] = nc.dram_tensor("dq", [P, S], F16, kind="ExternalOutput").ap()
        taps["dk"] = nc.dram_tensor("dk", [P, S], F16, kind="ExternalOutput").ap()
        taps["dv0"] = nc.dram_tensor("dv0", [P, 66 * NT], F32, kind="ExternalOutput").ap()
        taps["dv1"] = nc.dram_tensor("dv1", [P, 66 * NT], F32, kind="ExternalOutput").ap()
        for kk in range(4):
            taps[f"de{kk}"] = nc.dram_tensor(f"de{kk}", [P, 1024], F32, kind="ExternalOutput").ap()
        taps["dzt"] = nc.dram_tensor("dzt", [P, 264], F32, kind="ExternalOutput").ap()

    with tile.TileContext(nc) as tc:
        with tc.tile_pool(name="const", bufs=1) as cpool, \
             tc.tile_pool(name="big", bufs=1) as big:
            # selector [128, 2]: col h sums partitions of head h (for sum-of-squares)
            sel2 = cpool.tile([P, 2], F16, tag="sel2")
            nc.vector.memset(sel2[:], 0.0)
            nc.vector.memset(sel2[0:64, 0:1], 1.0)
            nc.vector.memset(sel2[64:128, 1:2], 1.0)
            # broadcast selector [2, 128]: bsel[p, i] = 1 iff i // 64 == p,
            # built as ones gated by two affine selects (64p <= i < 64p+64)
            bsel = cpool.tile([2, P], F32R, tag="bsel")
            bselt = cpool.tile([2, P], F32, tag="bselt")
            nc.vector.memset(bselt[:], 1.0)
            nc.gpsimd.affine_select(out=bselt[:], in_=bselt[:],
                                    compare_op=mybir.AluOpType.is_ge, fill=0.0,
                                    base=0, channel_multiplier=-64, pattern=[[1, P]])
            nc.gpsimd.affine_select(out=bselt[:], in_=bselt[:],
                                    compare_op=mybir.AluOpType.is_ge, fill=0.0,
                                    base=63, channel_multiplier=64, pattern=[[-1, P]])
            nc.gpsimd.tensor_copy(bsel[:], bselt[:])
            # act-table prefetch: one dummy exp then sqrt so the table loads
            # overlap the input DMA instead of the critical path
            warm = cpool.tile([1, 2], F32, tag="warm")
            nc.vector.memset(warm[:], 1.0)
            nc.scalar.activation(warm[:, 0:1], warm[:, 0:1], AF.Exp,
                                 bias=0.0, scale=1.0)
            nc.scalar.activation(warm[:, 1:2], warm[:, 1:2], AF.Sqrt,
                                 bias=0.0, scale=1.0)

            # persistent tiles
            xts = big.tile([P, NCH * S], F16, tag="xts")        # x^T din-chunks
            wts = big.tile([P, NCH * 3 * P], F16, tag="wts")    # w^T din-chunks
            qTn = big.tile([P, S], F16, tag="qTn")              # normalized q^T
            kTn = big.tile([P, S], F16, tag="kTn")              # normalized k^T
            # v augmented with a ones column per s-tile: [v_h (64) | 1]
            vaug = [big.tile([P, 66 * NT], BF16, name=f"vaug{h}", tag=f"vaug{h}")
                    for h in range(2)]

            for h in range(2):
                nc.vector.memset(vaug[h][:], 1.0)
            # per-q-chunk input DMAs (each delivers that chunk's columns of
            # every din chunk, so all its consumers share one completion
            # event); first projection starts after 1/4 of x has landed
            nc.sync.dma_start(wts[:], wt[:])
            for sc in range(NQC):
                cols = [slice(None), slice(None), slice(QC * sc, QC * (sc + 1))]
                nc.sync.dma_start(
                    xts[:].rearrange("p (c s) -> p c s", c=NCH)[tuple(cols)],
                    xt.rearrange("p (c s) -> p c s", c=NCH)[tuple(cols)])

            # ============ phase 1: projections + rmsnorm factors ============
            with tc.tile_pool(name="pps", bufs=2, space="PSUM") as pps, \
                 tc.tile_pool(name="sps", bufs=1, space="PSUM") as sps, \
                 tc.tile_pool(name="fps", bufs=1, space="PSUM") as fps, \
                 tc.tile_pool(name="vps", bufs=2, space="PSUM") as vps, \
                 tc.tile_pool(name="sq", bufs=2) as sqp, \
                 tc.tile_pool(name="fr", bufs=2) as frp:
                for sc in range(NQC):
                    qs = slice(QC * sc, QC * (sc + 1))
                    ssqk = sps.tile([2, 2 * QC], F32, tag="ssqk")
                    sq = sqp.tile([P, 2 * QC], F16, tag="sq")
                    raws = []
                    for ki, w0 in ((0, 0), (1, P)):  # q then k
                        ps = pps.tile([P, QC], F32, tag="proj")
                        for c in range(NCH):
                            nc.tensor.matmul(
                                ps[:],
                                wts[:, 3 * P * c + w0:3 * P * c + w0 + P],
                                xts[:, S * c + QC * sc:S * c + QC * (sc + 1)],
                                start=(c == 0), stop=(c == 3))
                        raw = sqp.tile([P, QC], F16, tag="raw")
                        nc.vector.tensor_copy(raw[:], ps[:])
                        nc.gpsimd.tensor_mul(sq[:, QC * ki:QC * (ki + 1)],
                                             raw[:], raw[:])
                        nc.tensor.matmul(ssqk[:, QC * ki:QC * (ki + 1)], sel2[:],
                                         sq[:, QC * ki:QC * (ki + 1)],
                                         start=True, stop=True)
                        raws.append(raw)
                    # factors: 1/sqrt(mean sq); rows = heads, cols = [fq | fk]
                    srt = frp.tile([2, 2 * QC], F32, tag="srt")
                    ftmp = frp.tile([2, 2 * QC], F32, tag="ftmp")
                    frec = frp.tile([2, 2 * QC], F32R, tag="frec")
                    nc.scalar.activation(srt[:], ssqk[:], AF.Sqrt,
                                         bias=0.0, scale=1.0 / DH)
                    nc.vector.reciprocal_approx_fast(ftmp[:], srt[:])
                    nc.gpsimd.tensor_copy(frec[:], ftmp[:])
                    # broadcast factor rows across dh partitions via PE
                    for ki, dst in ((0, qTn), (1, kTn)):
                        fb = fps.tile([P, QC], F32, tag="fb")
                        nc.tensor.matmul(fb[:], bsel[:],
                                         frec[:, QC * ki:QC * (ki + 1)],
                                         start=True, stop=True)
                        nc.vector.tensor_mul(dst[:, qs], raws[ki][:], fb[:])
                    # v in native orientation [s-pos, dh] with ones column
                    psv = vps.tile([P, QC], F32, tag="psv")
                    for tl in range(4):
                        t = 4 * sc + tl
                        for c in range(NCH):
                            nc.tensor.matmul(
                                psv[:, P * tl:P * (tl + 1)],
                                xts[:, S * c + P * t:S * c + P * (t + 1)],
                                wts[:, 3 * P * c + 2 * P:3 * P * (c + 1)],
                                start=(c == 0), stop=(c == 3))
                    for h in range(2):
                        for tl in range(4):
                            t = 4 * sc + tl
                            nc.vector.tensor_copy(
                                vaug[h][:, 66 * t:66 * t + 64],
                                psv[:, P * tl + 64 * h:P * tl + 64 * (h + 1)])

            # ================= phase 2: attention =================
            with tc.tile_pool(name="scps", bufs=2, space="PSUM") as scps, \
                 tc.tile_pool(name="ztps", bufs=2, space="PSUM") as ztps, \
                 tc.tile_pool(name="att", bufs=18) as att, \
                 tc.tile_pool(name="nrm", bufs=2) as nrm, \
                 tc.tile_pool(name="zo", bufs=2) as zop:
                for j in range(NQC):
                    q0 = QC * j
                    nkb = 4 * j + 4
                    zts = [ztps.tile([P, 264], F32, name=f"zt{j}_{h}", tag=f"zt{h}")
                           for h in range(2)]
                    eTs = []
                    for kb in range(nkb):
                        i = kb - 4 * j
                        if i == 3:
                            continue  # folded into the i == 2 tile below
                        qoff = max(0, P * i)
                        fr = QC - qoff
                        # head h occupies cols [512h : 512h + ...] so each
                        # matmul output stays inside one 512-float psum bank
                        sc_ps = scps.tile([P, 1024], F32, tag="sc")
                        eT = att.tile([P, 1024], BF16, tag="eT")
                        if i == 2:
                            # pack i=2 (256 cols) and i=3 (128 cols) side by
                            # side per head; one exp covers both blocks
                            for ii in range(2):
                                qo2 = P * (2 + ii)
                                w = QC - qo2
                                for h in range(2):
                                    hs = slice(64 * h, 64 * (h + 1))
                                    nc.tensor.matmul(
                                        sc_ps[:, QC * h + 256 * ii:
                                              QC * h + 256 * ii + w],
                                        kTn[hs, P * (kb + ii):P * (kb + ii + 1)],
                                        qTn[hs, q0 + qo2:q0 + QC],
                                        start=True, stop=True)
                            ap_o = eT[:].rearrange("p (h f) -> p h f", h=2)[:, :, 0:384]
                            ap_i = sc_ps[:].rearrange("p (h f) -> p h f", h=2)[:, :, 0:384]
                            nc.scalar.activation(ap_o, ap_i,
                                                 AF.Exp, bias=0.0, scale=1.0)
                            for off in (0, 256):
                                tri = eT[:].rearrange("p (h f) -> p h f", h=2)[:, :, off:off + P]
                                nc.gpsimd.affine_select(
                                    out=tri, in_=tri,
                                    compare_op=mybir.AluOpType.is_ge, fill=0.0,
                                    base=0, channel_multiplier=-1,
                                    pattern=[[0, 2], [1, P]])
                            eTs.append((256, 0, eT))    # i=2: qoff, base
                            eTs.append((384, 256, eT))  # i=3
                            continue
                        for h in range(2):
                            hs = slice(64 * h, 64 * (h + 1))
                            nc.tensor.matmul(sc_ps[:, QC * h:QC * h + fr],
                                             kTn[hs, P * kb:P * (kb + 1)],
                                             qTn[hs, q0 + qoff:q0 + QC],
                                             start=True, stop=True)
                        if fr == QC:
                            nc.scalar.activation(eT[:], sc_ps[:],
                                                 AF.Exp, bias=0.0, scale=1.0)
                        else:
                            ap_o = eT[:].rearrange("p (h f) -> p h f", h=2)[:, :, 0:fr]
                            ap_i = sc_ps[:].rearrange("p (h f) -> p h f", h=2)[:, :, 0:fr]
                            nc.scalar.activation(ap_o, ap_i,
                                                 AF.Exp, bias=0.0, scale=1.0)
                        if i >= 0:
                            # zero the strictly-upper triangle of the diagonal
                            # 128-col block of each head (keep where col >= p)
                            tri = eT[:].rearrange("p (h f) -> p h f", h=2)[:, :, 0:P]
                            nc.gpsimd.affine_select(
                                out=tri, in_=tri,
                                compare_op=mybir.AluOpType.is_ge, fill=0.0,
                                base=0, channel_multiplier=-1,
                                pattern=[[0, 2], [1, P]])
                        eTs.append((qoff, 0, eT))
                    # z accumulation: q on partitions, free = [v 64 | denom].
                    # qsub-major so each psum region's accumulation group
                    # closes before the next one starts (a psum bank holds
                    # only one open accumulation group at a time).
                    for qsub in range(4):
                        for h in range(2):
                            for kb in range(4 * j + qsub + 1):
                                qoff, base, eT = eTs[kb]
                                if P * qsub < qoff:
                                    continue
                                nc.tensor.matmul(
                                    zts[h][:, 66 * qsub:66 * (qsub + 1)],
                                    eT[:, base + QC * h + P * qsub - qoff:
                                          base + QC * h + P * qsub - qoff + P],
                                    vaug[h][:, 66 * kb:66 * (kb + 1)],
                                    start=(kb == 0), stop=(kb == 4 * j + qsub))
                    zsb = zop.tile([P, 4 * P], F16, tag="zsb")
                    for h in range(2):
                        rcp = nrm.tile([P, 4], F32, tag="rcp")
                        nc.vector.reciprocal(rcp[:], zts[h][:, 64::66])
                        for qsub in range(4):
                            nc.vector.tensor_scalar(
                                out=zsb[:, P * qsub + 64 * h:P * qsub + 64 * (h + 1)],
                                in0=zts[h][:, 66 * qsub:66 * qsub + 64],
                                scalar1=rcp[:, qsub:qsub + 1],
                                scalar2=None,
                                op0=mybir.AluOpType.mult)
                    nc.sync.dma_start(
                        z[q0:q0 + QC, :].rearrange("(qs p) c -> p qs c", p=P),
                        zsb[:].rearrange("p (qs c) -> p qs c", qs=4))
                    if debug_taps and j == 0:
                        zcopy = zop.tile([P, 264], F32, tag="zcopy")
                        nc.vector.tensor_copy(zcopy[:], zts[0][:])
                        nc.sync.dma_start(taps["dzt"], zcopy[:])
            if debug_taps:
                nc.sync.dma_start(taps["dq"], qTn[:])
                nc.sync.dma_start(taps["dk"], kTn[:])
                pass
                pass

    nc.finalize()
    return nc


def _numpy_reference(x, Wqkv, Wo, scale_q, scale_k, mask):
    b, s, d = x.shape
    dh = d // NH
    qkv = x @ Wqkv.T
    q, k, v = np.split(qkv, 3, axis=-1)

    def rms(t, scale):
        r = np.sqrt(np.mean(np.square(t), axis=-1, keepdims=True)) + EPS
        return t / r * scale

    q = rms(q.reshape(b, s, NH, dh), scale_q)
    k = rms(k.reshape(b, s, NH, dh), scale_k)
    v = v.reshape(b, s, NH, dh)
    attn = np.einsum('bqhd,bkhd->bhqk', q, k)
    attn = np.where(mask[None, None], -np.inf, attn)
    attn = attn - attn.max(axis=-1, keepdims=True)
    p = np.exp(attn)
    p = p / p.sum(axis=-1, keepdims=True)
    zz = np.einsum('bhqk,bkhd->bqhd', p, v).reshape(b, s, d)
    return (zz @ Wo.T).astype(np.float32)


class _Runner:
    """Caches the jitted shard_map executable for nc across calls."""

    def __init__(self, nc, n_cores=8):
        import jax
        import concourse.mybir as mybir
        from jax.sharding import Mesh, PartitionSpec
        from jax.experimental.shard_map import shard_map
        from concourse.bass2jax import (
            _bass_exec_p, install_neuronx_cc_hook, partition_id_tensor)

        install_neuronx_cc_hook()
        self.nc = nc
        self.n_cores = n_cores
        partition_name = nc.partition_id_tensor.name if nc.partition_id_tensor else None
        in_names, out_names, out_avals, zero_shapes = [], [], [], []
        for alloc in nc.m.functions[0].allocations:
            if not isinstance(alloc, mybir.MemoryLocationSet):
                continue
            name = alloc.memorylocations[0].name
            if alloc.kind == "ExternalInput":
                if name != partition_name:
                    in_names.append(name)
            elif alloc.kind == "ExternalOutput":
                out_names.append(name)
                shape = tuple(alloc.tensor_shape)
                dtype = mybir.dt.np(alloc.dtype)
                out_avals.append(jax.core.ShapedArray(shape, dtype))
                zero_shapes.append((shape, dtype))
        self.in_names = in_names
        self.out_names = out_names
        self.zero_shapes = zero_shapes
        n_params = len(in_names)
        n_outs = len(out_avals)
        in_names_all = in_names + out_names + ([partition_name] if partition_name else [])
        donate = tuple(range(n_params, n_params + n_outs))

        def _body(*args):
            operands = list(args)
            if partition_name is not None:
                operands.append(partition_id_tensor())
            outs = _bass_exec_p.bind(
                *operands, out_avals=tuple(out_avals),
                in_names=tuple(in_names_all), out_names=tuple(out_names),
                lowering_input_output_aliases=(),
                sim_require_finite=True, sim_require_nnan=True, nc=nc)
            return tuple(outs)

        devices = jax.devices()[:n_cores]
        self.mesh = Mesh(np.asarray(devices), ("core",))
        in_specs = (PartitionSpec("core"),) * (n_params + n_outs)
        out_specs = (PartitionSpec("core"),) * n_outs
        self.sharded = jax.jit(
            shard_map(_body, mesh=self.mesh, in_specs=in_specs,
                      out_specs=out_specs, check_rep=False),
            donate_argnums=donate, keep_unused=True)
        self._zmaker = jax.jit(
            lambda: tuple(
                jax.numpy.zeros((n_cores * sh[0], *sh[1:]), dt)
                for sh, dt in zero_shapes),
            out_shardings=tuple(
                jax.sharding.NamedSharding(self.mesh, PartitionSpec("core"))
                for _ in zero_shapes))

    def concat_inputs(self, in_maps):
        return [np.concatenate([np.asarray(m[nm]) for m in in_maps], axis=0)
                for nm in self.in_names]

    def run_concat(self, concat_in):
        """concat_in: list of (n_cores*dim0, ...) arrays (np or device)."""
        zeros = self._zmaker()
        outs = self.sharded(*concat_in, *zeros)
        return outs

    def fetch(self, outs):
        res = []
        for c in range(self.n_cores):
            d = {}
            for i, name in enumerate(self.out_names):
                sh = self.zero_shapes[i][0]
                d[name] = np.asarray(outs[i]).reshape(self.n_cores, *sh)[c]
            res.append(d)
        return res


def _get_runner():
    if "runner" not in _CACHE:
        _CACHE["runner"] = _Runner(_build_program())
    return _CACHE["runner"]


def _prep_inputs(x, Wqkv):
    """Build per-core device inputs: fp16, transposed, din-chunk-major.

    xt: [128, 4*2048] where chunk c cols hold x[b].T rows 128c:128c+128.
    wt: [128, 4*384]  where chunk c holds [WqT | WkT | WvT] rows of chunk c.
    """
    xt = x.transpose(0, 2, 1).astype(np.float16)           # [2, 512, 2048]
    xtc = [np.ascontiguousarray(
        xt[bb].reshape(NCH, P, S).transpose(1, 0, 2).reshape(P, NCH * S))
        for bb in range(B)]
    w = Wqkv.reshape(3, D, D)
    in_maps = []
    for core in range(8):
        bb = core // 4
        g = core % 4
        r = slice(P * g, P * (g + 1))
        wt = np.concatenate([w[0][r].T, w[1][r].T, w[2][r].T],
                            axis=1).astype(np.float16)      # [512, 384]
        wtc = wt.reshape(NCH, P, 3 * P).transpose(1, 0, 2).reshape(P, NCH * 3 * P)
        in_maps.append({"xt": xtc[bb], "wt": np.ascontiguousarray(wtc)})
    return in_maps


def kernel(x, Wqkv, Wo, scale_q, scale_k, mask):
    x = np.asarray(x, np.float32)
    Wqkv = np.asarray(Wqkv, np.float32)
    Wo = np.asarray(Wo, np.float32)
    causal = np.triu(np.ones((S, S), dtype=bool), k=1)
    if (not np.allclose(np.asarray(scale_q), 1.0) or
            not np.allclose(np.asarray(scale_k), 1.0) or
            not np.array_equal(np.asarray(mask), causal) or
            x.shape != (B, S, D)):
        return _numpy_reference(x, Wqkv, Wo, np.asarray(scale_q), np.asarray(scale_k),
                                np.asarray(mask))

    runner = _get_runner()
    in_maps = _prep_inputs(x, Wqkv)
    _CACHE["last_in_maps"] = in_maps
    concat_in = runner.concat_inputs(in_maps)
    outs = runner.run_concat(concat_in)
    res = runner.fetch(outs)
    # host epilogue: out[b] = concat_g(z_g) @ Wo^T
    outp = np.empty((B, S, D), np.float32)
    WoT = np.ascontiguousarray(Wo.T)
    for bb in range(B):
        zb = np.concatenate([res[4 * bb + g]["z"] for g in range(4)],
                            axis=1).astype(np.float32)     # [2048, 512]
        np.dot(zb, WoT, out=outp[bb])
    return outp


# revision 43
# speedup vs baseline: 3751.9645x; 1.0651x over previous
"""Trainium2 Bass kernel for nn_CausalVideoAttention (b=2, s=2048, d=512, 8 heads).

Sharding: 8 cores = (batch, head-pair): core c -> batch c//4, heads {2*(c%4), 2*(c%4)+1}.
Each core computes the qkv projection for its head pair (fp16 inputs, f32 psum),
rms-normalizes q and k (factors folded in on-chip), runs causal attention over the
full sequence, and returns normalized per-pair z in native [seq, dh*2] orientation
(fp16). The host applies the output projection Wo (one sgemm per batch) and
concatenates head pairs. No device collectives.

On-chip layout: activations stored [feature, seq] ("transposed domain") so every
projection/score matmul contracts over partitions; the z accumulation is done with
q on partitions (out free = 65 = dh+denominator) which halves PE row count vs the
wide-free form and makes softmax normalization a per-partition scalar multiply.
"""

import sys

for _p in ("/opt/trn_rl_repo",):
    if _p not in sys.path:
        sys.path.insert(0, _p)

import numpy as np

B, S, D = 2, 2048, 512
NH, DH = 8, 64
P = 128          # partitions / tile edge
NCH = D // P     # 4 din chunks
QC = 512         # q-chunk width
NQC = S // QC    # 4 q-chunks
NT = S // P      # 16 s-tiles
EPS = 1e-6

_CACHE = {}


def _build_program(debug_taps=False):
    import concourse.bass as bass  # noqa: F401
    import concourse.mybir as mybir
    import concourse.tile as tile
    from concourse import bacc

    F32 = mybir.dt.float32
    F32R = mybir.dt.float32r
    F16 = mybir.dt.float16
    BF16 = mybir.dt.bfloat16
    AF = mybir.ActivationFunctionType

    nc = bacc.Bacc("TRN2", target_bir_lowering=False, debug=False, num_devices=8)
    xt = nc.dram_tensor("xt", [P, NCH * S], F16, kind="ExternalInput").ap()
    wt = nc.dram_tensor("wt", [P, NCH * 3 * P], F16, kind="ExternalInput").ap()
    z = nc.dram_tensor("z", [S, P], F16, kind="ExternalOutput").ap()
    taps = {}
    if debug_taps:
        taps["dq"] = nc.dram_tensor("dq", [P, S], F16, kind="ExternalOutput").ap()
        taps["dk"] = nc.dram_tensor("dk", [P, S], F16, kind="ExternalOutput").ap()
        taps["dv0"] = nc.dram_tensor("dv0", [P, 66 * NT], F32, kind="ExternalOutput").ap()
        taps["dv1"] = nc.dram_tensor("dv1", [P, 66 * NT], F32, kind="ExternalOutput").ap()
        for kk in range(4):
            taps[f"de{kk}"] = nc.dram_tensor(f"de{kk}", [P, 1024], F32, kind="ExternalOutput").ap()
        taps["dzt"] = nc.dram_tensor("dzt", [P, 264], F32, kind="ExternalOutput").ap()

    with tile.TileContext(nc) as tc:
        with tc.tile_pool(name="const", bufs=1) as cpool, \
             tc.tile_pool(name="big", bufs=1) as big:
            # selector [128, 2]: col h sums partitions of head h (for sum-of-squares)
            sel2 = cpool.tile([P, 2], F16, tag="sel2")
            nc.vector.memset(sel2[:], 0.0)
            nc.vector.memset(sel2[0:64, 0:1], 1.0)
            nc.vector.memset(sel2[64:128, 1:2], 1.0)
            # broadcast selector [2, 128]: bsel[p, i] = 1 iff i // 64 == p,
            # built as ones gated by two affine selects (64p <= i < 64p+64)
            bsel = cpool.tile([2, P], F32R, tag="bsel")
            bselt = cpool.tile([2, P], F32, tag="bselt")
            nc.vector.memset(bselt[:], 1.0)
            nc.gpsimd.affine_select(out=bselt[:], in_=bselt[:],
                                    compare_op=mybir.AluOpType.is_ge, fill=0.0,
                                    base=0, channel_multiplier=-64, pattern=[[1, P]])
            nc.gpsimd.affine_select(out=bselt[:], in_=bselt[:],
                                    compare_op=mybir.AluOpType.is_ge, fill=0.0,
                                    base=63, channel_multiplier=64, pattern=[[-1, P]])
            nc.gpsimd.tensor_copy(bsel[:], bselt[:])
            # k-side copy of the selector at partition base 32 (cross-
            # partition move is only possible via DMA; f32r bits preserved)
            bselk = cpool.tile([34, P], F32R, tag="bselk")
            nc.sync.dma_start(bselk[32:34, :], bsel[:])
            # act-table prefetch: one dummy exp then sqrt so the table loads
            # overlap the input DMA instead of the critical path
            warm = cpool.tile([1, 2], F32, tag="warm")
            nc.vector.memset(warm[:], 1.0)
            nc.scalar.activation(warm[:, 0:1], warm[:, 0:1], AF.Exp,
                                 bias=0.0, scale=1.0)
            nc.scalar.activation(warm[:, 1:2], warm[:, 1:2], AF.Sqrt,
                                 bias=0.0, scale=1.0)

            # persistent tiles
            xts = big.tile([P, NCH * S], F16, tag="xts")        # x^T din-chunks
            wts = big.tile([P, NCH * 3 * P], F16, tag="wts")    # w^T din-chunks
            qTn = big.tile([P, S], F16, tag="qTn")              # normalized q^T
            kTn = big.tile([P, S], F16, tag="kTn")              # normalized k^T
            # v augmented with a ones column per s-tile: [v_h (64) | 1]
            vaug = [big.tile([P, 66 * NT], BF16, name=f"vaug{h}", tag=f"vaug{h}")
                    for h in range(2)]

            for h in range(2):
                nc.vector.memset(vaug[h][:], 1.0)
            # per-q-chunk input DMAs (each delivers that chunk's columns of
            # every din chunk, so all its consumers share one completion
            # event); first projection starts after 1/4 of x has landed
            nc.sync.dma_start(wts[:], wt[:])
            for sc in range(NQC):
                cols = [slice(None), slice(None), slice(QC * sc, QC * (sc + 1))]
                nc.sync.dma_start(
                    xts[:].rearrange("p (c s) -> p c s", c=NCH)[tuple(cols)],
                    xt.rearrange("p (c s) -> p c s", c=NCH)[tuple(cols)])

            # ============ phase 1: projections + rmsnorm factors ============
            with tc.tile_pool(name="pps", bufs=2, space="PSUM") as pps, \
                 tc.tile_pool(name="sps", bufs=1, space="PSUM") as sps, \
                 tc.tile_pool(name="fps", bufs=1, space="PSUM") as fps, \
                 tc.tile_pool(name="vps", bufs=2, space="PSUM") as vps, \
                 tc.tile_pool(name="sq", bufs=2) as sqp, \
                 tc.tile_pool(name="fr", bufs=2) as frp:
                for sc in range(NQC):
                    qs = slice(QC * sc, QC * (sc + 1))
                    # q head-sums at partitions 0:2, k at 32:34 so one
                    # 512-col sqrt/recip covers both (partitions are free)
                    ssqk = sps.tile([34, QC], F32, tag="ssqk")
                    sq = sqp.tile([P, 2 * QC], F16, tag="sq")
                    raws = []
                    for ki, w0 in ((0, 0), (1, P)):  # q then k
                        ps = pps.tile([P, QC], F32, tag="proj")
                        for c in range(NCH):
                            nc.tensor.matmul(
                                ps[:],
                                wts[:, 3 * P * c + w0:3 * P * c + w0 + P],
                                xts[:, S * c + QC * sc:S * c + QC * (sc + 1)],
                                start=(c == 0), stop=(c == 3))
                        raw = sqp.tile([P, QC], F16, tag="raw")
                        nc.vector.tensor_copy(raw[:], ps[:])
                        nc.gpsimd.tensor_mul(sq[:, QC * ki:QC * (ki + 1)],
                                             raw[:], raw[:])
                        nc.tensor.matmul(ssqk[32 * ki:32 * ki + 2, :], sel2[:],
                                         sq[:, QC * ki:QC * (ki + 1)],
                                         start=True, stop=True)
                        raws.append(raw)
                    # factors: 1/sqrt(mean sq); rows 0:2 fq, rows 32:34 fk
                    # (rows 2:32 are untouched psum garbage, never read)
                    srt = frp.tile([34, QC], F32, tag="srt")
                    ftmp = frp.tile([34, QC], F32, tag="ftmp")
                    frec = frp.tile([34, QC], F32R, tag="frec")
                    nc.scalar.activation(srt[:], ssqk[:], AF.Sqrt,
                                         bias=0.0, scale=1.0 / DH)
                    nc.vector.reciprocal_approx_fast(ftmp[:], srt[:])
                    nc.gpsimd.tensor_copy(frec[:], ftmp[:])
                    if sc == NQC - 1:
                        # trigger the exp-table reload now (depends on the
                        # last sqrt) so it overlaps phase-1 tail work
                        # instead of stalling the first attention exp
                        nc.scalar.activation(warm[:, 0:1], srt[0:1, 0:1],
                                             AF.Exp, bias=0.0, scale=1.0)
                    # broadcast factor rows across dh partitions via PE
                    for ki, dst in ((0, qTn), (1, kTn)):
                        fb = fps.tile([P, QC], F32, tag="fb")
                        nc.tensor.matmul(fb[:],
                                         bsel[:] if ki == 0 else bselk[32:34, :],
                                         frec[32 * ki:32 * ki + 2, :],
                                         start=True, stop=True)
                        nc.vector.tensor_mul(dst[:, qs], raws[ki][:], fb[:])
                    # v in native orientation [s-pos, dh] with ones column
                    psv = vps.tile([P, QC], F32, tag="psv")
                    for tl in range(4):
                        t = 4 * sc + tl
                        for c in range(NCH):
                            nc.tensor.matmul(
                                psv[:, P * tl:P * (tl + 1)],
                                xts[:, S * c + P * t:S * c + P * (t + 1)],
                                wts[:, 3 * P * c + 2 * P:3 * P * (c + 1)],
                                start=(c == 0), stop=(c == 3))
                    for h in range(2):
                        for tl in range(4):
                            t = 4 * sc + tl
                            nc.vector.tensor_copy(
                                vaug[h][:, 66 * t:66 * t + 64],
                                psv[:, P * tl + 64 * h:P * tl + 64 * (h + 1)])

            # ================= phase 2: attention =================
            with tc.tile_pool(name="scps", bufs=2, space="PSUM") as scps, \
                 tc.tile_pool(name="ztps", bufs=2, space="PSUM") as ztps, \
                 tc.tile_pool(name="att", bufs=18) as att, \
                 tc.tile_pool(name="nrm", bufs=2) as nrm, \
                 tc.tile_pool(name="zo", bufs=2) as zop:
                for j in range(NQC):
                    q0 = QC * j
                    nkb = 4 * j + 4
                    zts = [ztps.tile([P, 264], F32, name=f"zt{j}_{h}", tag=f"zt{h}")
                           for h in range(2)]
                    eTs = []
                    for kb in range(nkb):
                        i = kb - 4 * j
                        if i == 3:
                            continue  # folded into the i == 2 tile below
                        qoff = max(0, P * i)
                        fr = QC - qoff
                        # head h occupies cols [512h : 512h + ...] so each
                        # matmul output stays inside one 512-float psum bank
                        sc_ps = scps.tile([P, 1024], F32, tag="sc")
                        eT = att.tile([P, 1024], BF16, tag="eT")
                        if i == 2:
                            # pack i=2 (256 cols) and i=3 (128 cols) side by
                            # side per head; one exp covers both blocks
                            for ii in range(2):
                                qo2 = P * (2 + ii)
                                w = QC - qo2
                                for h in range(2):
                                    hs = slice(64 * h, 64 * (h + 1))
                                    nc.tensor.matmul(
                                        sc_ps[:, QC * h + 256 * ii:
                                              QC * h + 256 * ii + w],
                                        kTn[hs, P * (kb + ii):P * (kb + ii + 1)],
                                        qTn[hs, q0 + qo2:q0 + QC],
                                        start=True, stop=True)
                            ap_o = eT[:].rearrange("p (h f) -> p h f", h=2)[:, :, 0:384]
                            ap_i = sc_ps[:].rearrange("p (h f) -> p h f", h=2)[:, :, 0:384]
                            nc.scalar.activation(ap_o, ap_i,
                                                 AF.Exp, bias=0.0, scale=1.0)
                            for off in (0, 256):
                                tri = eT[:].rearrange("p (h f) -> p h f", h=2)[:, :, off:off + P]
                                nc.gpsimd.affine_select(
                                    out=tri, in_=tri,
                                    compare_op=mybir.AluOpType.is_ge, fill=0.0,
                                    base=0, channel_multiplier=-1,
                                    pattern=[[0, 2], [1, P]])
                            eTs.append((256, 0, eT))    # i=2: qoff, base
                            eTs.append((384, 256, eT))  # i=3
                            continue
                        for h in range(2):
                            hs = slice(64 * h, 64 * (h + 1))
                            nc.tensor.matmul(sc_ps[:, QC * h:QC * h + fr],
                                             kTn[hs, P * kb:P * (kb + 1)],
                                             qTn[hs, q0 + qoff:q0 + QC],
                                             start=True, stop=True)
                        if fr == QC:
                            nc.scalar.activation(eT[:], sc_ps[:],
                                                 AF.Exp, bias=0.0, scale=1.0)
                        else:
                            ap_o = eT[:].rearrange("p (h f) -> p h f", h=2)[:, :, 0:fr]
                            ap_i = sc_ps[:].rearrange("p (h f) -> p h f", h=2)[:, :, 0:fr]
                            nc.scalar.activation(ap_o, ap_i,
                                                 AF.Exp, bias=0.0, scale=1.0)
                        if i >= 0:
                            # zero the strictly-upper triangle of the diagonal
                            # 128-col block of each head (keep where col >= p)
                            tri = eT[:].rearrange("p (h f) -> p h f", h=2)[:, :, 0:P]
                            nc.gpsimd.affine_select(
                                out=tri, in_=tri,
                                compare_op=mybir.AluOpType.is_ge, fill=0.0,
                                base=0, channel_multiplier=-1,
                                pattern=[[0, 2], [1, P]])
                        eTs.append((qoff, 0, eT))
                    # z accumulation: q on partitions, free = [v 64 | denom].
                    # qsub-major so each psum region's accumulation group
                    # closes before the next one starts (a psum bank holds
                    # only one open accumulation group at a time).
                    for qsub in range(4):
                        for h in range(2):
                            for kb in range(4 * j + qsub + 1):
                                qoff, base, eT = eTs[kb]
                                if P * qsub < qoff:
                                    continue
                                nc.tensor.matmul(
                                    zts[h][:, 66 * qsub:66 * (qsub + 1)],
                                    eT[:, base + QC * h + P * qsub - qoff:
                                          base + QC * h + P * qsub - qoff + P],
                                    vaug[h][:, 66 * kb:66 * (kb + 1)],
                                    start=(kb == 0), stop=(kb == 4 * j + qsub))
                    zsb = zop.tile([P, 4 * P], F16, tag="zsb")
                    for h in range(2):
                        rcp = nrm.tile([P, 4], F32, tag="rcp")
                        nc.vector.reciprocal(rcp[:], zts[h][:, 64::66])
                        for qsub in range(4):
                            nc.vector.tensor_scalar(
                                out=zsb[:, P * qsub + 64 * h:P * qsub + 64 * (h + 1)],
                                in0=zts[h][:, 66 * qsub:66 * qsub + 64],
                                scalar1=rcp[:, qsub:qsub + 1],
                                scalar2=None,
                                op0=mybir.AluOpType.mult)
                    nc.sync.dma_start(
                        z[q0:q0 + QC, :].rearrange("(qs p) c -> p qs c", p=P),
                        zsb[:].rearrange("p (qs c) -> p qs c", qs=4))
                    if debug_taps and j == 0:
                        zcopy = zop.tile([P, 264], F32, tag="zcopy")
                        nc.vector.tensor_copy(zcopy[:], zts[0][:])
                        nc.sync.dma_start(taps["dzt"], zcopy[:])
            if debug_taps:
                nc.sync.dma_start(taps["dq"], qTn[:])
                nc.sync.dma_start(taps["dk"], kTn[:])
                pass
                pass

    nc.finalize()
    return nc


def _numpy_reference(x, Wqkv, Wo, scale_q, scale_k, mask):
    b, s, d = x.shape
    dh = d // NH
    qkv = x @ Wqkv.T
    q, k, v = np.split(qkv, 3, axis=-1)

    def rms(t, scale):
        r = np.sqrt(np.mean(np.square(t), axis=-1, keepdims=True)) + EPS
        return t / r * scale

    q = rms(q.reshape(b, s, NH, dh), scale_q)
    k = rms(k.reshape(b, s, NH, dh), scale_k)
    v = v.reshape(b, s, NH, dh)
    attn = np.einsum('bqhd,bkhd->bhqk', q, k)
    attn = np.where(mask[None, None], -np.inf, attn)
    attn = attn - attn.max(axis=-1, keepdims=True)
    p = np.exp(attn)
    p = p / p.sum(axis=-1, keepdims=True)
    zz = np.einsum('bhqk,bkhd->bqhd', p, v).reshape(b, s, d)
    return (zz @ Wo.T).astype(np.float32)


class _Runner:
    """Caches the jitted shard_map executable for nc across calls."""

    def __init__(self, nc, n_cores=8):
        import jax
        import concourse.mybir as mybir
        from jax.sharding import Mesh, PartitionSpec
        from jax.experimental.shard_map import shard_map
        from concourse.bass2jax import (
            _bass_exec_p, install_neuronx_cc_hook, partition_id_tensor)

        install_neuronx_cc_hook()
        self.nc = nc
        self.n_cores = n_cores
        partition_name = nc.partition_id_tensor.name if nc.partition_id_tensor else None
        in_names, out_names, out_avals, zero_shapes = [], [], [], []
        for alloc in nc.m.functions[0].allocations:
            if not isinstance(alloc, mybir.MemoryLocationSet):
                continue
            name = alloc.memorylocations[0].name
            if alloc.kind == "ExternalInput":
                if name != partition_name:
                    in_names.append(name)
            elif alloc.kind == "ExternalOutput":
                out_names.append(name)
                shape = tuple(alloc.tensor_shape)
                dtype = mybir.dt.np(alloc.dtype)
                out_avals.append(jax.core.ShapedArray(shape, dtype))
                zero_shapes.append((shape, dtype))
        self.in_names = in_names
        self.out_names = out_names
        self.zero_shapes = zero_shapes
        n_params = len(in_names)
        n_outs = len(out_avals)
        in_names_all = in_names + out_names + ([partition_name] if partition_name else [])
        donate = tuple(range(n_params, n_params + n_outs))

        def _body(*args):
            operands = list(args)
            if partition_name is not None:
                operands.append(partition_id_tensor())
            outs = _bass_exec_p.bind(
                *operands, out_avals=tuple(out_avals),
                in_names=tuple(in_names_all), out_names=tuple(out_names),
                lowering_input_output_aliases=(),
                sim_require_finite=True, sim_require_nnan=True, nc=nc)
            return tuple(outs)

        devices = jax.devices()[:n_cores]
        self.mesh = Mesh(np.asarray(devices), ("core",))
        in_specs = (PartitionSpec("core"),) * (n_params + n_outs)
        out_specs = (PartitionSpec("core"),) * n_outs
        self.sharded = jax.jit(
            shard_map(_body, mesh=self.mesh, in_specs=in_specs,
                      out_specs=out_specs, check_rep=False),
            donate_argnums=donate, keep_unused=True)
        self._zmaker = jax.jit(
            lambda: tuple(
                jax.numpy.zeros((n_cores * sh[0], *sh[1:]), dt)
                for sh, dt in zero_shapes),
            out_shardings=tuple(
                jax.sharding.NamedSharding(self.mesh, PartitionSpec("core"))
                for _ in zero_shapes))

    def concat_inputs(self, in_maps):
        return [np.concatenate([np.asarray(m[nm]) for m in in_maps], axis=0)
                for nm in self.in_names]

    def run_concat(self, concat_in):
        """concat_in: list of (n_cores*dim0, ...) arrays (np or device)."""
        zeros = self._zmaker()
        outs = self.sharded(*concat_in, *zeros)
        return outs

    def fetch(self, outs):
        res = []
        for c in range(self.n_cores):
            d = {}
            for i, name in enumerate(self.out_names):
                sh = self.zero_shapes[i][0]
                d[name] = np.asarray(outs[i]).reshape(self.n_cores, *sh)[c]
            res.append(d)
        return res


def _get_runner():
    if "runner" not in _CACHE:
        _CACHE["runner"] = _Runner(_build_program())
    return _CACHE["runner"]


def _prep_inputs(x, Wqkv):
    """Build per-core device inputs: fp16, transposed, din-chunk-major.

    xt: [128, 4*2048] where chunk c cols hold x[b].T rows 128c:128c+128.
    wt: [128, 4*384]  where chunk c holds [WqT | WkT | WvT] rows of chunk c.
    """
    xt = x.transpose(0, 2, 1).astype(np.float16)           # [2, 512, 2048]
    xtc = [np.ascontiguousarray(
        xt[bb].reshape(NCH, P, S).transpose(1, 0, 2).reshape(P, NCH * S))
        for bb in range(B)]
    w = Wqkv.reshape(3, D, D)
    in_maps = []
    for core in range(8):
        bb = core // 4
        g = core % 4
        r = slice(P * g, P * (g + 1))
        wt = np.concatenate([w[0][r].T, w[1][r].T, w[2][r].T],
                            axis=1).astype(np.float16)      # [512, 384]
        wtc = wt.reshape(NCH, P, 3 * P).transpose(1, 0, 2).reshape(P, NCH * 3 * P)
        in_maps.append({"xt": xtc[bb], "wt": np.ascontiguousarray(wtc)})
    return in_maps


def kernel(x, Wqkv, Wo, scale_q, scale_k, mask):
    x = np.asarray(x, np.float32)
    Wqkv = np.asarray(Wqkv, np.float32)
    Wo = np.asarray(Wo, np.float32)
    causal = np.triu(np.ones((S, S), dtype=bool), k=1)
    if (not np.allclose(np.asarray(scale_q), 1.0) or
            not np.allclose(np.asarray(scale_k), 1.0) or
            not np.array_equal(np.asarray(mask), causal) or
            x.shape != (B, S, D)):
        return _numpy_reference(x, Wqkv, Wo, np.asarray(scale_q), np.asarray(scale_k),
                                np.asarray(mask))

    runner = _get_runner()
    in_maps = _prep_inputs(x, Wqkv)
    _CACHE["last_in_maps"] = in_maps
    concat_in = runner.concat_inputs(in_maps)
    outs = runner.run_concat(concat_in)
    res = runner.fetch(outs)
    # host epilogue: out[b] = concat_g(z_g) @ Wo^T
    outp = np.empty((B, S, D), np.float32)
    WoT = np.ascontiguousarray(Wo.T)
    for bb in range(B):
        zb = np.concatenate([res[4 * bb + g]["z"] for g in range(4)],
                            axis=1).astype(np.float32)     # [2048, 512]
        np.dot(zb, WoT, out=outp[bb])
    return outp
